# revision 15
# baseline (speedup 1.0000x reference)
"""Trainium2 Bass kernel for nn_DiscreteDiffusionActionHead.

Strategy: pure data-parallel over batch (B=8 -> 1 element per NeuronCore,
no collectives). All activations held in [dim(partitions), token(free)]
layout. bf16 matmul inputs with fp32 PSUM accumulation.

Host-side (free, not on HW critical path):
  - token-embedding gather, proprio projection, rope cos/sin tables
  - fold attention scale into Wq/bq, tanh(gate) into Wkt/bkt
  - pad q/k projection output dims per head (112 -> 128) so each head owns
    one partition tile; pad Wo input dim to match
  - rot_half expressed as a [128,128] shift matrix applied on the PE
  - downcast + pre-tile all weights/hidden-states into exact SBUF layouts
"""
import numpy as np
import ml_dtypes

BF16 = ml_dtypes.bfloat16
F16 = np.float16
F32 = np.float32
WSCALE = 256.0

L_FULL = 24
D = 896
NH = 8
HD = 112
HP = 128
MQ = NH * HP            # 1024
KT = D // 128           # 7
KTO = MQ // 128         # 8
T = 56
NVIS = 512
NADP = 64
NA = NADP + 1           # 65
VOCAB = 256
PD = 8
EPS = 1e-5
NCORES = 8

# trig pack offsets (free-dim columns)
TRIG_W = 2 * (T + NA + NVIS)  # 1266
O_CQ, O_SQ = 0, T
O_CA, O_SA = 2 * T, 2 * T + NA
O_CT, O_ST = 2 * T + 2 * NA, 2 * T + 2 * NA + NVIS

# bias pack offsets (per-partition [128, 60])
B_Q, B_KS, B_KA, B_KT = 0, 8, 16, 24
B_O, B_F, B_G, B_B = 32, 39, 46, 53
NBP = 60


# ----------------------------------------------------------------------------
# host-side layout helpers
# ----------------------------------------------------------------------------

def _rope_tables(n):
    inv = 1.0 / (10000.0 ** (np.arange(0, HD, 2, dtype=F32) / HD))
    f = np.arange(n, dtype=F32)[:, None] * inv[None, :]
    emb = np.concatenate([f, f], axis=-1)               # (n, 112)
    return np.cos(emb), np.sin(emb)


def _trig_pad(n):
    c, s = _rope_tables(n)
    cp = np.zeros((HP, n), F32)
    sp = np.zeros((HP, n), F32)
    cp[:HD] = c.T
    sp[:HD] = s.T
    return cp, sp


def _pad_cols(W):
    Wp = np.zeros((W.shape[0], MQ), F32)
    for h in range(NH):
        Wp[:, HP * h:HP * h + HD] = W[:, HD * h:HD * h + HD]
    return Wp


def _pad_rows(W):
    Wp = np.zeros((MQ, W.shape[1]), F32)
    for h in range(NH):
        Wp[HP * h:HP * h + HD, :] = W[HD * h:HD * h + HD, :]
    return Wp


def _pad_vec(b):
    bp = np.zeros(MQ, F32)
    for h in range(NH):
        bp[HP * h:HP * h + HD] = b[HD * h:HD * h + HD]
    return bp


def _lhsT(W, dtype=F16):
    """[Din, M] -> [128, Din//128, M] sbuf layout."""
    Din, M = W.shape
    return np.ascontiguousarray(
        W.reshape(Din // 128, 128, M).transpose(1, 0, 2)).astype(dtype)


def _pk(b):
    """per-partition bias pack: [nm*128] -> [128, nm]"""
    nm = b.shape[0] // 128
    return np.ascontiguousarray(b.reshape(nm, 128).T).astype(F32)


def _shift_T():
    S = np.zeros((HP, HP), F32)
    for i in range(HD // 2):
        S[2 * i, 2 * i + 1] = -1.0
        S[2 * i + 1, 2 * i] = 1.0
    return np.ascontiguousarray(S.T).astype(F16)


def prep_shared(inp, L):
    """Layout transforms shared by all cores (weights etc)."""
    g = {}
    for k, v in inp.items():
        a = np.asarray(v)
        g[k] = a if np.issubdtype(a.dtype, np.integer) else a.astype(F32)
    scale = F32(1.0 / np.sqrt(HD))
    rg = np.tanh(g['gate'])                      # [L]

    wq = np.empty((L, 128, KT, MQ), F16)
    wks = np.empty((L, 128, KT, MQ), F16)
    wka = np.empty((L, 128, KT, MQ), F16)
    wkt = np.empty((L, 128, KT, MQ), F16)
    wvs = np.empty((L, 128, KT, D), F16)
    wva = np.empty((L, 128, KT, D), F16)
    wvt = np.empty((L, 128, KT, D), F16)
    wo = np.empty((L, 128, KTO, D), F32)
    wf = np.empty((L, 128, KT, D), F32)
    bias_pk = np.empty((L, 128, NBP), F32)
    bias_v = np.empty((L, 1, 3, D), F32)

    for l in range(L):
        wq[l] = _lhsT(_pad_cols(g['Wq'][l] * (scale * WSCALE)))
        wks[l] = _lhsT(_pad_cols(g['Wks'][l] * WSCALE))
        wka[l] = _lhsT(_pad_cols(g['Wka'][l] * WSCALE))
        wkt[l] = _lhsT(_pad_cols(g['Wkt'][l] * (rg[l] * WSCALE)))
        wvs[l] = _lhsT(g['Wvs'][l] * WSCALE)
        wva[l] = _lhsT(g['Wva'][l] * WSCALE)
        wvt[l] = _lhsT(g['Wvt'][l] * WSCALE)
        wo[l] = _lhsT(_pad_rows(g['Wo'][l]), F32)
        wf[l] = _lhsT(g['Wf'][l], F32)
        bias_pk[l, :, B_Q:B_Q + 8] = _pk(_pad_vec(g['bq'][l] * scale))
        bias_pk[l, :, B_KS:B_KS + 8] = _pk(_pad_vec(g['bks'][l]))
        bias_pk[l, :, B_KA:B_KA + 8] = _pk(_pad_vec(g['bka'][l]))
        bias_pk[l, :, B_KT:B_KT + 8] = _pk(_pad_vec(g['bkt'][l] * rg[l]))
        bias_pk[l, :, B_O:B_O + 7] = _pk(g['bo'][l])
        bias_pk[l, :, B_F:B_F + 7] = _pk(g['bf'][l])
        bias_pk[l, :, B_G:B_G + 7] = _pk(g['ln_g'][l])
        bias_pk[l, :, B_B:B_B + 7] = _pk(g['ln_b'][l])
        bias_v[l, 0, 0] = g['bvs'][l]
        bias_v[l, 0, 1] = g['bva'][l]
        bias_v[l, 0, 2] = g['bvt'][l]

    trig = np.empty((128, TRIG_W), F32)
    trig[:, O_CQ:O_CQ + T], trig[:, O_SQ:O_SQ + T] = _trig_pad(T)
    trig[:, O_CA:O_CA + NA], trig[:, O_SA:O_SA + NA] = _trig_pad(NA)
    trig[:, O_CT:O_CT + NVIS], trig[:, O_ST:O_ST + NVIS] = _trig_pad(NVIS)

    fin = np.zeros((128, 16), F32)
    fin[:, 0:7] = _pk(g['og'])
    fin[:, 7:14] = _pk(g['ob'])
    fin[:, 14:16] = _pk(g['bout'])

    shared = {
        'wq': wq, 'wks': wks, 'wka': wka, 'wkt': wkt,
        'wvs': wvs, 'wva': wva, 'wvt': wvt, 'wo': wo, 'wf': wf,
        'wout': _lhsT(g['Wout'], F32),
        'bias_pk': bias_pk, 'bias_v': bias_v,
        'trig': trig.astype(F16), 'fin': fin, 'shift_t': _shift_T(),
    }
    return shared, g


def prep_core(g, b, L):
    """Per-core (= per batch element) activations in sbuf layout."""
    mhs = g['multi_layer_hidden_states']
    p = g['proprio'][b] @ g['Wp'] + g['bp']                    # [D]
    ht = np.empty((L, 128, KT, NVIS), F16)
    ha = np.empty((L, 128, KT, NA), F16)
    for l in range(L):
        ht[l] = mhs[b, l, :NVIS, :].T.reshape(KT, 128, NVIS).transpose(1, 0, 2)
        ha_full = np.concatenate([mhs[b, l, NVIS:, :], p[None]], 0).T  # [D, 65]
        ha[l] = ha_full.reshape(KT, 128, NA).transpose(1, 0, 2)
    x0 = g['tok_emb'][np.asarray(g['input_tokens'][b], np.int64)].T   # [D, T]
    x0 = np.ascontiguousarray(x0.reshape(KT, 128, T).transpose(1, 0, 2)).astype(F32)
    return {'ht': ht, 'ha': ha, 'x0': x0}


# ----------------------------------------------------------------------------
# bass program
# ----------------------------------------------------------------------------

def build_program(L, xdbg=False, variant=None):
    import itertools
    _ctr = itertools.count()
    import concourse.tile as tile
    import concourse.mybir as mybir
    from concourse import bacc

    dt = mybir.dt
    AF = mybir.ActivationFunctionType
    OP = mybir.AluOpType
    DS = 1.0 / WSCALE

    nc = bacc.Bacc("TRN2", target_bir_lowering=False, debug=False,
                   num_devices=NCORES, name="ddah")

    def din(name, shape, dtype=dt.float16):
        return nc.dram_tensor(name, shape, dtype, kind="ExternalInput")

    d_wq = din("wq", [L, 128, KT, MQ])
    d_wks = din("wks", [L, 128, KT, MQ])
    d_wka = din("wka", [L, 128, KT, MQ])
    d_wkt = din("wkt", [L, 128, KT, MQ])
    d_wvs = din("wvs", [L, 128, KT, D])
    d_wva = din("wva", [L, 128, KT, D])
    d_wvt = din("wvt", [L, 128, KT, D])
    d_wo = din("wo", [L, 128, KTO, D], dt.float32)
    d_wf = din("wf", [L, 128, KT, D], dt.float32)
    d_wout = din("wout", [128, KT, VOCAB], dt.float32)
    d_bpk = din("bias_pk", [L, 128, NBP], dt.float32)
    d_bv = din("bias_v", [L, 1, 3, D], dt.float32)
    d_trig = din("trig", [128, TRIG_W])
    d_fin = din("fin", [128, 16], dt.float32)
    d_shift = din("shift_t", [128, 128])
    d_ht = din("ht", [L, 128, KT, NVIS])
    d_ha = din("ha", [L, 128, KT, NA])
    d_x0 = din("x0", [128, KT, T], dt.float32)
    d_out = nc.dram_tensor("out", [128, 2, T], dt.float32, kind="ExternalOutput")
    d_xdbg = None
    taps = {}

    def tap(name, tile_, l=0):
        if not xdbg or l != 0 or name in taps:
            return
        dtt = nc.dram_tensor(f"tap_{name}", list(tile_.shape), tile_.dtype,
                             kind="ExternalOutput")
        taps[name] = dtt
        nc.sync.dma_start(dtt[:], tile_)
    if xdbg:
        d_xdbg = nc.dram_tensor("xdbg", [L, 128, KT, T], dt.float32,
                                kind="ExternalOutput")

    MH0, MH1 = 4, 3   # Wo/Wf fp32 m-tile halves: tiles 0-3 (512 cols), 4-6 (384)

    kvb = 1 if variant == "noweightdma" else 2
    with tile.TileContext(nc) as tc, \
         tc.tile_pool(name="singles", bufs=1) as singles, \
         tc.tile_pool(name="wp16", bufs=3) as wp16, \
         tc.tile_pool(name="wp32", bufs=2) as wp32, \
         tc.tile_pool(name="iop", bufs=kvb) as iop, \
         tc.tile_pool(name="kvp", bufs=kvb) as kvp, \
         tc.tile_pool(name="tmp", bufs=kvb) as tmp, \
         tc.tile_pool(name="att", bufs=kvb) as att, \
         tc.tile_pool(name="xp", bufs=kvb * 2) as xp, \
         tc.tile_pool(name="yp", bufs=kvb) as yp, \
         tc.tile_pool(name="st", bufs=2 * kvb) as st, \
         tc.tile_pool(name="bvp", bufs=1) as bvp, \
         tc.tile_pool(name="ps1", bufs=4, space="PSUM") as ps1, \
         tc.tile_pool(name="ps2", bufs=2, space="PSUM") as ps2:

        # ---- constants loaded once ----
        trig = singles.tile([128, TRIG_W], dt.float16, name="trig")
        nc.sync.dma_start(trig, d_trig[:])
        shift = singles.tile([128, 128], dt.float16, name="shift")
        nc.sync.dma_start(shift, d_shift[:])
        fin = singles.tile([128, 16], dt.float32, name="fin")
        nc.sync.dma_start(fin, d_fin[:])
        ones_h = singles.tile([128, 1], dt.float16, name="ones_h")
        nc.vector.memset(ones_h, 1.0)
        ones_f = singles.tile([128, 1], dt.float32, name="ones_f")
        nc.vector.memset(ones_f, 1.0)
        eps_t = singles.tile([1, 1], dt.float32, name="eps_t")
        nc.vector.memset(eps_t, EPS)
        ones_row = singles.tile([1, 128], dt.float32, name="ones_row")
        nc.vector.memset(ones_row, 1.0)

        cq, sq = trig[:, O_CQ:O_CQ + T], trig[:, O_SQ:O_SQ + T]
        ca, sa = trig[:, O_CA:O_CA + NA], trig[:, O_SA:O_SA + NA]
        ct, stg = trig[:, O_CT:O_CT + NVIS], trig[:, O_ST:O_ST + NVIS]

        x_sb = xp.tile([128, KT, T], dt.float32, tag="x", name="x_init")
        nc.sync.dma_start(x_sb, d_x0[:])

        hoist = (variant == "noweightdma")
        dma_only = (variant == "dmaonly")
        _hoisted = {}

        def load_w(dram, l, ktiles, m):
            if hoist:
                key = id(dram)
                if key not in _hoisted:
                    w = singles.tile([128, ktiles, m], dt.float16,
                                     name=f"hw{next(_ctr)}")
                    nc.sync.dma_start(w, dram[0])
                    _hoisted[key] = w
                return _hoisted[key]
            w = wp16.tile([128, ktiles, m], dt.float16, tag="w",
                          name=f"t{next(_ctr)}")
            nc.sync.dma_start(w, dram[l])
            return w

        def load_w32(dram, l, ktiles, half):
            cols = slice(0, 512) if half == 0 else slice(512, D)
            n = 512 if half == 0 else D - 512
            if hoist:
                key = id(dram)
                if key not in _hoisted:
                    w = singles.tile([128, ktiles, 512], dt.float32,
                                     name=f"hw{next(_ctr)}")
                    nc.sync.dma_start(w, dram[0][:, :, 0:512])
                    _hoisted[key] = w
                return _hoisted[key][:, :, :n]
            w = wp32.tile([128, ktiles, n], dt.float32, tag="w32",
                          name=f"t{next(_ctr)}")
            nc.sync.dma_start(w, dram[l][:, :, cols])
            return w

        def proj_qk(w_sb, rhs_sb, n, bias_col, cos, sin, bsb):
            """fp16 padded-head projection + rope. Returns [128, NH, n] fp16."""
            out = kvp.tile([128, NH, n], dt.float16,
                           tag=f"qk{n}_{bias_col}", name=f"t{next(_ctr)}")
            for h in range(NH):
                ps = ps1.tile([128, 512], dt.float32, tag="ps1",
                              name=f"t{next(_ctr)}")[:, :n]
                for k in range(KT):
                    nc.tensor.matmul(ps, w_sb[:, k, HP * h:HP * h + HP],
                                     rhs_sb[:, k, :],
                                     start=(k == 0), stop=(k == KT - 1))
                ksb = tmp.tile([128, NVIS], dt.float16, tag="rk",
                               name=f"t{next(_ctr)}")[:, :n]
                nc.vector.tensor_scalar(ksb, ps, DS,
                                        bsb[:, bias_col + h:bias_col + h + 1],
                                        OP.mult, OP.add)
                rps = ps1.tile([128, 512], dt.float32, tag="ps1",
                               name=f"t{next(_ctr)}")[:, :n]
                nc.tensor.matmul(rps, shift, ksb, start=True, stop=True)
                t1 = tmp.tile([128, NVIS], dt.float16, tag="rt1",
                              name=f"t{next(_ctr)}")[:, :n]
                nc.vector.tensor_tensor(t1, ksb, cos, OP.mult)
                t2 = tmp.tile([128, NVIS], dt.float16, tag="rt2",
                              name=f"t{next(_ctr)}")[:, :n]
                nc.vector.tensor_tensor(t2, rps, sin, OP.mult)
                nc.vector.tensor_tensor(out[:, h, :], t1, t2, OP.add)
            return out

        def proj_v(rhs_w, act_sb, mtiles, mlast, tag, bv_row, bvsb):
            """fp16 swapped projection -> [tok, D] layout."""
            out = kvp.tile([128, mtiles, D], dt.float16, tag=f"v{tag}",
                           name=f"t{next(_ctr)}") \
                if mtiles > 1 else \
                kvp.tile([mlast, 1, D], dt.float16, tag=f"v{tag}",
                         name=f"t{next(_ctr)}")
            for m in range(mtiles):
                mw = 128 if m < mtiles - 1 else mlast
                ps = ps2.tile([128, D], dt.float32, tag="ps2",
                              name=f"t{next(_ctr)}")[:mw]
                for sl in (slice(0, 512), slice(512, D)):
                    for k in range(KT):
                        nc.tensor.matmul(
                            ps[:, sl],
                            act_sb[:, k, 128 * m:128 * m + mw],
                            rhs_w[:, k, sl],
                            start=(k == 0), stop=(k == KT - 1))
                nc.vector.scalar_tensor_tensor(
                    out[:mw, m, :], ps, DS, bvsb[:mw, bv_row, :],
                    OP.mult, OP.add)
            return out

        nrep = 1
        if isinstance(variant, tuple) and variant[0] == "rep":
            nrep = variant[1]
        if variant == "empty":
            nrep = 0
        for li in range(L * nrep):
            l = li % L
            if isinstance(variant, str) and variant.startswith("probe_"):
                which = variant[6:]
                if li == 0:
                    sink = singles.tile([128, 1], dt.float32, name="sink")
                if which == "bv":
                    bvsb = bvp.tile([128, 3, D], dt.float32, tag="biasv",
                                    name=f"t{next(_ctr)}")
                    nc.gpsimd.dma_start(bvsb, d_bv[l].to_broadcast((128, 3, D)))
                    nc.vector.tensor_copy(out=sink, in_=bvsb[:, 0, 0:1])
                elif which == "bias":
                    bsb = st.tile([128, NBP], dt.float32, tag="bias",
                                  name=f"t{next(_ctr)}")
                    nc.sync.dma_start(bsb, d_bpk[l])
                    nc.vector.tensor_copy(out=sink, in_=bsb[:, 0:1])
                elif which == "ht":
                    ht_sb = iop.tile([128, KT, NVIS], dt.float16, tag="ht",
                                     name=f"t{next(_ctr)}")
                    nc.sync.dma_start(ht_sb, d_ht[l])
                    nc.vector.tensor_copy(out=sink, in_=ht_sb[:, 0, 0:1])
                elif which == "ha":
                    ha_sb = iop.tile([128, KT, NA], dt.float16, tag="ha",
                                     name=f"t{next(_ctr)}")
                    nc.sync.dma_start(ha_sb, d_ha[l])
                    nc.vector.tensor_copy(out=sink, in_=ha_sb[:, 0, 0:1])
                elif which == "w":
                    w = load_w(d_wq, l, KT, MQ)
                    nc.vector.tensor_copy(out=sink, in_=w[:, 0, 0:1])
                elif which == "w4":
                    for dram in (d_wq, d_wks, d_wka, d_wkt):
                        w = load_w(dram, l, KT, MQ)
                        nc.vector.tensor_copy(out=sink, in_=w[:, 0, 0:1])
                elif which == "wall7":
                    for dram, m in ((d_wq, MQ), (d_wks, MQ), (d_wka, MQ),
                                    (d_wkt, MQ), (d_wvs, D), (d_wva, D),
                                    (d_wvt, D)):
                        w = load_w(dram, l, KT, m)
                        nc.vector.tensor_copy(out=sink, in_=w[:, 0, 0:1])
                continue
            bsb = st.tile([128, NBP], dt.float32, tag="bias",
                          name=f"t{next(_ctr)}")
            nc.sync.dma_start(bsb, d_bpk[l])
            bvsb = bvp.tile([128, 3, D],
                            dt.float16 if hoist else dt.float32, tag="biasv",
                            name=f"t{next(_ctr)}")
            nc.gpsimd.dma_start(bvsb, d_bv[l].to_broadcast((128, 3, D)))
            ht_sb = iop.tile([128, KT, NVIS], dt.float16, tag="ht",
                             name=f"t{next(_ctr)}")
            nc.sync.dma_start(ht_sb, d_ht[l])
            ha_sb = iop.tile([128, KT, NA], dt.float16, tag="ha",
                             name=f"t{next(_ctr)}")
            nc.sync.dma_start(ha_sb, d_ha[l])

            if variant in ("dmabig2", "dmabig4"):
                # split the big transfer across multiple DGE queues
                if li == 0:
                    sink = singles.tile([128, 1], dt.float32, name="sink")
                    WALL = 9556
                    d_wall = nc.dram_tensor("wall", [L, 128, WALL], dt.float16,
                                            kind="ExternalInput")
                w = wp16.tile([128, 9556], dt.float16, tag="wall",
                              name=f"t{next(_ctr)}")
                if variant == "dmabig2":
                    nc.sync.dma_start(w[:, :4778], d_wall[l][:, :4778])
                    nc.gpsimd.dma_start(w[:, 4778:], d_wall[l][:, 4778:])
                else:
                    nc.sync.dma_start(w[:, :3185], d_wall[l][:, :3185])
                    nc.scalar.dma_start(w[:, 3185:6370], d_wall[l][:, 3185:6370])
                    nc.gpsimd.dma_start(w[:, 6370:], d_wall[l][:, 6370:])
                nc.vector.tensor_copy(out=sink, in_=w[:, 0:1])
                continue

            if variant in ("dmabig", "dmabig_sw"):
                # single huge DMA per layer, same total bytes as all weights
                if li == 0:
                    sink = singles.tile([128, 1], dt.float32, name="sink")
                    WALL = 9556  # fp16 elems/partition ~= 19.1MB total
                    d_wall = nc.dram_tensor("wall", [L, 128, WALL], dt.float16,
                                            kind="ExternalInput")
                w = wp16.tile([128, 9556], dt.float16, tag="wall",
                              name=f"t{next(_ctr)}")
                if variant == "dmabig_sw":
                    nc.gpsimd.dma_start(w, d_wall[l])
                else:
                    nc.sync.dma_start(w, d_wall[l])
                nc.vector.tensor_copy(out=sink, in_=w[:, 0:1])
                continue

            if dma_only:
                if li == 0:
                    sink = singles.tile([128, 1], dt.float32, name="sink")
                for dram, kt_, m in ((d_wkt, KT, MQ), (d_wvt, KT, D),
                                     (d_wka, KT, MQ), (d_wva, KT, D),
                                     (d_wq, KT, MQ), (d_wks, KT, MQ),
                                     (d_wvs, KT, D)):
                    w = load_w(dram, l, kt_, m)
                    nc.vector.tensor_copy(out=sink, in_=w[:, 0, 0:1])
                for half in (0, 1):
                    w = load_w32(d_wo, l, KTO, half)
                    nc.vector.tensor_copy(out=sink, in_=w[:, 0, 0:1])
                    w = load_w32(d_wf, l, KT, half)
                    nc.vector.tensor_copy(out=sink, in_=w[:, 0, 0:1])
                nc.vector.tensor_copy(out=sink, in_=ht_sb[:, 0, 0:1])
                nc.vector.tensor_copy(out=sink, in_=ha_sb[:, 0, 0:1])
                continue

            # fp16 shadow of the fp32 residual stream
            x16 = xp.tile([128, KT, T], dt.float16, tag="x16",
                          name=f"t{next(_ctr)}")
            nc.vector.tensor_copy(out=x16, in_=x_sb)

            # ---- kv path ----
            w = load_w(d_wkt, l, KT, MQ)
            kt_ro = proj_qk(w, ht_sb, NVIS, B_KT, ct, stg, bsb)
            tap("kt_ro", kt_ro, l)
            w = load_w(d_wvt, l, KT, D)
            vt_sb = proj_v(w, ht_sb, 4, 128, "t", 2, bvsb)
            tap("vt", vt_sb, l)
            w = load_w(d_wka, l, KT, MQ)
            ka_ro = proj_qk(w, ha_sb, NA, B_KA, ca, sa, bsb)
            w = load_w(d_wva, l, KT, D)
            va_sb = proj_v(w, ha_sb, 1, NA, "a", 1, bvsb)

            # ---- x projections ----
            w = load_w(d_wq, l, KT, MQ)
            q_ro = proj_qk(w, x16, T, B_Q, cq, sq, bsb)
            tap("q_ro", q_ro, l)
            w = load_w(d_wks, l, KT, MQ)
            ks_ro = proj_qk(w, x16, T, B_KS, cq, sq, bsb)
            w = load_w(d_wvs, l, KT, D)
            vs_sb = proj_v(w, x16, 1, T, "s", 0, bvsb)

            # ---- attention (scores transposed: [keys, queries]) ----
            NT = NH * T  # 448
            ex_s = att.tile([T, NT], dt.float16, tag="exs",
                            name=f"t{next(_ctr)}")
            ex_a = att.tile([NA, NT], dt.float16, tag="exa",
                            name=f"t{next(_ctr)}")
            ex_t = att.tile([128, 4, NT], dt.float16, tag="ext",
                            name=f"t{next(_ctr)}")

            ps = ps1.tile([T, 512], dt.float32, tag="ps1",
                          name=f"t{next(_ctr)}")[:, :NT]
            for h in range(NH):
                nc.tensor.matmul(ps[:, T * h:T * h + T], ks_ro[:, h, :],
                                 q_ro[:, h, :], start=True, stop=True)
            nc.scalar.activation(ex_s, ps, AF.Exp)
            tap("ex_s", ex_s, l)
            ps = ps1.tile([NA, 512], dt.float32, tag="ps1",
                          name=f"t{next(_ctr)}")[:, :NT]
            for h in range(NH):
                nc.tensor.matmul(ps[:, T * h:T * h + T], ka_ro[:, h, :],
                                 q_ro[:, h, :], start=True, stop=True)
            nc.scalar.activation(ex_a, ps, AF.Exp)
            for m in range(4):
                ps = ps1.tile([128, 512], dt.float32, tag="ps1",
                              name=f"t{next(_ctr)}")[:, :NT]
                for h in range(NH):
                    nc.tensor.matmul(ps[:, T * h:T * h + T],
                                     kt_ro[:, h, 128 * m:128 * m + 128],
                                     q_ro[:, h, :], start=True, stop=True)
                nc.scalar.activation(ex_t[:, m, :], ps, AF.Exp)

            lps = ps1.tile([1, 512], dt.float32, tag="ps1",
                           name=f"t{next(_ctr)}")[:, :NT]
            nc.tensor.matmul(lps, ones_h[:T], ex_s, start=True, stop=False)
            nc.tensor.matmul(lps, ones_h[:NA], ex_a, start=False, stop=False)
            for m in range(4):
                nc.tensor.matmul(lps, ones_h, ex_t[:, m, :],
                                 start=False, stop=(m == 3))
            linv = st.tile([1, NT], dt.float32, tag="linv",
                           name=f"t{next(_ctr)}")
            nc.vector.reciprocal(linv, lps)
            tap("linv", linv, l)
            lrep_ps = ps1.tile([128, 512], dt.float32, tag="ps1",
                               name=f"t{next(_ctr)}")[:, :NT]
            nc.tensor.matmul(lrep_ps, ones_row, linv, start=True, stop=True)
            linv_b = att.tile([128, NT], dt.float32, tag="linvb",
                              name=f"t{next(_ctr)}")
            nc.vector.tensor_copy(out=linv_b, in_=lrep_ps)

            o_pad = att.tile([128, NH, T], dt.float32, tag="opad",
                             name=f"t{next(_ctr)}")
            nc.vector.memset(o_pad[96:128], 0.0)
            for h in range(NH):
                hs = slice(HD * h, HD * h + HD)
                ops = ps1.tile([HD, 512], dt.float32, tag="ps1",
                               name=f"t{next(_ctr)}")[:, :T]
                nc.tensor.matmul(ops, vs_sb[:T, 0, hs], ex_s[:, T * h:T * h + T],
                                 start=True, stop=False)
                nc.tensor.matmul(ops, va_sb[:NA, 0, hs], ex_a[:, T * h:T * h + T],
                                 start=False, stop=False)
                for m in range(4):
                    nc.tensor.matmul(ops, vt_sb[:, m, hs],
                                     ex_t[:, m, T * h:T * h + T],
                                     start=False, stop=(m == 3))
                nc.vector.tensor_tensor(
                    o_pad[:HD, h, :], ops, linv_b[:HD, T * h:T * h + T], OP.mult)
            tap("o_pad", o_pad, l)

            # ---- Wo (fp32) + residual ----
            y_sb = yp.tile([128, KT, T], dt.float32, tag="y",
                           name=f"t{next(_ctr)}")
            if variant == "wobf16":
                wq16 = load_w(d_wq, l, KT, MQ)
                o16 = att.tile([128, NH, T], dt.float16, tag="o16",
                               name=f"t{next(_ctr)}")
                nc.vector.tensor_copy(out=o16, in_=o_pad)
                for mo in range(KT):
                    ps = ps1.tile([128, 512], dt.float32, tag="ps1",
                                  name=f"t{next(_ctr)}")[:, :T]
                    for k in range(KTO):
                        nc.tensor.matmul(ps, wq16[:, k % KT, 128 * mo:128 * mo + 128],
                                         o16[:, k, :],
                                         start=(k == 0), stop=(k == KTO - 1))
                    nc.vector.scalar_tensor_tensor(
                        y_sb[:, mo, :], ps, bsb[:, B_O + mo:B_O + mo + 1],
                        x_sb[:, mo, :], OP.add, OP.add)
            else:
                for half, mos in ((0, range(4)), (1, range(4, KT))):
                    w32 = load_w32(d_wo, l, KTO, half)
                    for mo in mos:
                        mc = mo * 128 - half * 512
                        ps = ps1.tile([128, 512], dt.float32, tag="ps1",
                                      name=f"t{next(_ctr)}")[:, :T]
                        for k in range(KTO):
                            nc.tensor.matmul(ps, w32[:, k, mc:mc + 128],
                                             o_pad[:, k, :],
                                             start=(k == 0), stop=(k == KTO - 1))
                        nc.vector.scalar_tensor_tensor(
                            y_sb[:, mo, :], ps, bsb[:, B_O + mo:B_O + mo + 1],
                            x_sb[:, mo, :], OP.add, OP.add)
            tap("y", y_sb, l)

            # ---- layernorm (fp32) ----
            mps = ps1.tile([1, 512], dt.float32, tag="ps1",
                           name=f"t{next(_ctr)}")[:, :T]
            for k in range(KT):
                nc.tensor.matmul(mps, ones_f, y_sb[:, k, :],
                                 start=(k == 0), stop=(k == KT - 1))
            ysq = yp.tile([128, KT, T], dt.float32, tag="ysq",
                          name=f"t{next(_ctr)}")
            nc.scalar.activation(ysq, y_sb, AF.Square)
            sps = ps1.tile([1, 512], dt.float32, tag="ps1",
                           name=f"t{next(_ctr)}")[:, :T]
            for k in range(KT):
                nc.tensor.matmul(sps, ones_f, ysq[:, k, :],
                                 start=(k == 0), stop=(k == KT - 1))
            mean = st.tile([1, T], dt.float32, tag="mean",
                           name=f"t{next(_ctr)}")
            nc.vector.tensor_scalar_mul(mean, mps, 1.0 / D)
            msq = st.tile([1, T], dt.float32, tag="msq", name=f"t{next(_ctr)}")
            nc.vector.tensor_tensor(msq, mean, mean, OP.mult)
            var = st.tile([1, T], dt.float32, tag="var", name=f"t{next(_ctr)}")
            nc.vector.scalar_tensor_tensor(var, sps, 1.0 / D, msq,
                                           OP.mult, OP.subtract)
            rc = st.tile([1, 2 * T], dt.float32, tag="rc", name=f"t{next(_ctr)}")
            nc.scalar.activation(rc[:, :T], var, AF.Sqrt, bias=eps_t)
            nc.vector.reciprocal(rc[:, :T], rc[:, :T])
            nc.vector.tensor_tensor(rc[:, T:], mean, rc[:, :T], OP.mult)
            rrep_ps = ps1.tile([128, 512], dt.float32, tag="ps1",
                               name=f"t{next(_ctr)}")[:, :2 * T]
            nc.tensor.matmul(rrep_ps, ones_row, rc, start=True, stop=True)
            rc_b = st.tile([128, 2 * T], dt.float32, tag="rcb",
                           name=f"t{next(_ctr)}")
            nc.vector.tensor_copy(out=rc_b, in_=rrep_ps)
            lnf = yp.tile([128, KT, T], dt.float32, tag="lnf",
                          name=f"t{next(_ctr)}")
            for k in range(KT):
                t1 = tmp.tile([128, T], dt.float32, tag="lt1",
                              name=f"t{next(_ctr)}")
                nc.vector.tensor_tensor(t1, y_sb[:, k, :], rc_b[:, :T], OP.mult)
                t2 = tmp.tile([128, T], dt.float32, tag="lt2",
                              name=f"t{next(_ctr)}")
                nc.vector.tensor_tensor(t2, t1, rc_b[:, T:], OP.subtract)
                nc.vector.tensor_scalar(lnf[:, k, :], t2,
                                        bsb[:, B_G + k:B_G + k + 1],
                                        bsb[:, B_B + k:B_B + k + 1],
                                        OP.mult, OP.add)
            tap("ln", lnf, l)

            # ---- Wf (fp32) + relu -> new x ----
            x_new = xp.tile([128, KT, T], dt.float32, tag="x",
                            name=f"t{next(_ctr)}")
            if variant == "wobf16":
                wf16 = load_w(d_wks, l, KT, MQ)
                ln16 = att.tile([128, KT, T], dt.float16, tag="ln16",
                                name=f"t{next(_ctr)}")
                nc.vector.tensor_copy(out=ln16, in_=lnf)
                for mo in range(KT):
                    ps = ps1.tile([128, 512], dt.float32, tag="ps1",
                                  name=f"t{next(_ctr)}")[:, :T]
                    for k in range(KT):
                        nc.tensor.matmul(ps, wf16[:, k, 128 * mo:128 * mo + 128],
                                         ln16[:, k, :],
                                         start=(k == 0), stop=(k == KT - 1))
                    nc.scalar.activation(x_new[:, mo, :], ps, AF.Relu,
                                         bias=bsb[:, B_F + mo:B_F + mo + 1])
            else:
                for half, mos in ((0, range(4)), (1, range(4, KT))):
                    w32 = load_w32(d_wf, l, KT, half)
                    for mo in mos:
                        mc = mo * 128 - half * 512
                        ps = ps1.tile([128, 512], dt.float32, tag="ps1",
                                      name=f"t{next(_ctr)}")[:, :T]
                        for k in range(KT):
                            nc.tensor.matmul(ps, w32[:, k, mc:mc + 128],
                                             lnf[:, k, :],
                                             start=(k == 0), stop=(k == KT - 1))
                        nc.scalar.activation(x_new[:, mo, :], ps, AF.Relu,
                                             bias=bsb[:, B_F + mo:B_F + mo + 1])
            x_sb = x_new
            if d_xdbg is not None and li == l:
                nc.sync.dma_start(d_xdbg[l], x_sb)

        # ---- final layernorm + Wout (fp32) ----
        mps = ps1.tile([1, 512], dt.float32, tag="ps1",
                       name=f"t{next(_ctr)}")[:, :T]
        for k in range(KT):
            nc.tensor.matmul(mps, ones_f, x_sb[:, k, :],
                             start=(k == 0), stop=(k == KT - 1))
        ysq = yp.tile([128, KT, T], dt.float32, tag="ysq", name=f"t{next(_ctr)}")
        nc.scalar.activation(ysq, x_sb, AF.Square)
        sps = ps1.tile([1, 512], dt.float32, tag="ps1",
                       name=f"t{next(_ctr)}")[:, :T]
        for k in range(KT):
            nc.tensor.matmul(sps, ones_f, ysq[:, k, :],
                             start=(k == 0), stop=(k == KT - 1))
        mean = st.tile([1, T], dt.float32, tag="mean", name=f"t{next(_ctr)}")
        nc.vector.tensor_scalar_mul(mean, mps, 1.0 / D)
        msq = st.tile([1, T], dt.float32, tag="msq", name=f"t{next(_ctr)}")
        nc.vector.tensor_tensor(msq, mean, mean, OP.mult)
        var = st.tile([1, T], dt.float32, tag="var", name=f"t{next(_ctr)}")
        nc.vector.scalar_tensor_tensor(var, sps, 1.0 / D, msq,
                                       OP.mult, OP.subtract)
        rc = st.tile([1, 2 * T], dt.float32, tag="rc", name=f"t{next(_ctr)}")
        nc.scalar.activation(rc[:, :T], var, AF.Sqrt, bias=eps_t)
        nc.vector.reciprocal(rc[:, :T], rc[:, :T])
        nc.vector.tensor_tensor(rc[:, T:], mean, rc[:, :T], OP.mult)
        rrep_ps = ps1.tile([128, 512], dt.float32, tag="ps1",
                           name=f"t{next(_ctr)}")[:, :2 * T]
        nc.tensor.matmul(rrep_ps, ones_row, rc, start=True, stop=True)
        rc_b = st.tile([128, 2 * T], dt.float32, tag="rcb", name=f"t{next(_ctr)}")
        nc.vector.tensor_copy(out=rc_b, in_=rrep_ps)
        lnf = yp.tile([128, KT, T], dt.float32, tag="lnf", name=f"t{next(_ctr)}")
        for k in range(KT):
            t1 = tmp.tile([128, T], dt.float32, tag="lt1", name=f"t{next(_ctr)}")
            nc.vector.tensor_tensor(t1, x_sb[:, k, :], rc_b[:, :T], OP.mult)
            t2 = tmp.tile([128, T], dt.float32, tag="lt2", name=f"t{next(_ctr)}")
            nc.vector.tensor_tensor(t2, t1, rc_b[:, T:], OP.subtract)
            nc.vector.tensor_scalar(lnf[:, k, :], t2,
                                    fin[:, k:k + 1], fin[:, 7 + k:8 + k],
                                    OP.mult, OP.add)
        woutsb = wp32.tile([128, KT, VOCAB], dt.float32, tag="w32",
                           name="woutsb")
        nc.sync.dma_start(woutsb, d_wout[:])
        out_sb = yp.tile([128, 2, T], dt.float32, tag="outsb",
                         name=f"t{next(_ctr)}")
        for mo in range(2):
            ps = ps1.tile([128, 512], dt.float32, tag="ps1",
                          name=f"t{next(_ctr)}")[:, :T]
            for k in range(KT):
                nc.tensor.matmul(ps, woutsb[:, k, 128 * mo:128 * mo + 128],
                                 lnf[:, k, :],
                                 start=(k == 0), stop=(k == KT - 1))
            nc.vector.tensor_scalar_add(out_sb[:, mo, :], ps,
                                        fin[:, 14 + mo:15 + mo])
        nc.sync.dma_start(d_out[:], out_sb)

    nc.compile()
    return nc


_PROG_CACHE = {}


def _get_program(L, xdbg=False, variant=None):
    key = (L, xdbg, variant)
    if key not in _PROG_CACHE:
        _PROG_CACHE[key] = build_program(L, xdbg, variant)
    return _PROG_CACHE[key]


def run(inputs, L=L_FULL, xdbg=False):
    from concourse.bass_utils import run_bass_kernel_spmd
    nc = _get_program(L, xdbg)
    shared, g = prep_shared(inputs, L)
    in_maps = []
    for b in range(NCORES):
        m = dict(shared)
        m.update(prep_core(g, b, L))
        in_maps.append(m)
    res = run_bass_kernel_spmd(nc, in_maps, core_ids=list(range(NCORES)))
    outs = []
    for r in res.results:
        o = r["out"]                                    # [128, 2, T]
        outs.append(np.ascontiguousarray(o.transpose(2, 1, 0)).reshape(T, VOCAB))
    full = np.stack(outs).astype(F32)                   # [B, T, VOCAB]
    if xdbg:
        return full, res.results
    return full


def kernel(**inputs) -> np.ndarray:
    return run(inputs, L=L_FULL)



# revision 26
# speedup vs baseline: 1.1783x; 1.1783x over previous
"""Trainium2 Bass kernel for nn_DiscreteDiffusionActionHead.

Strategy: pure data-parallel over batch (B=8 -> 1 element per NeuronCore,
no collectives). All activations held in [dim(partitions), token(free)]
layout. bf16 matmul inputs with fp32 PSUM accumulation.

Host-side (free, not on HW critical path):
  - token-embedding gather, proprio projection, rope cos/sin tables
  - fold attention scale into Wq/bq, tanh(gate) into Wkt/bkt
  - pad q/k projection output dims per head (112 -> 128) so each head owns
    one partition tile; pad Wo input dim to match
  - rot_half expressed as a [128,128] shift matrix applied on the PE
  - downcast + pre-tile all weights/hidden-states into exact SBUF layouts
"""
import numpy as np
import ml_dtypes

BF16 = ml_dtypes.bfloat16
F16 = np.float16
F32 = np.float32
WSCALE = 256.0

L_FULL = 24
D = 896
NH = 8
HD = 112
HP = 128
MQ = NH * HP            # 1024
KT = D // 128           # 7
KTO = MQ // 128         # 8
T = 56
NVIS = 512
NADP = 64
NA = NADP + 1           # 65
VOCAB = 256
PD = 8
EPS = 1e-5
NCORES = 8

# trig pack offsets (free-dim columns)
TRIG_W = 2 * (T + NA + NVIS)  # 1266
O_CQ, O_SQ = 0, T
O_CA, O_SA = 2 * T, 2 * T + NA
O_CT, O_ST = 2 * T + 2 * NA, 2 * T + 2 * NA + NVIS

# bias pack offsets (per-partition [128, 60])
B_Q, B_KS, B_KA, B_KT = 0, 8, 16, 24
B_O, B_F, B_G, B_B = 32, 39, 46, 53
NBP = 60


# ----------------------------------------------------------------------------
# host-side layout helpers
# ----------------------------------------------------------------------------

def _rope_tables(n):
    inv = 1.0 / (10000.0 ** (np.arange(0, HD, 2, dtype=F32) / HD))
    f = np.arange(n, dtype=F32)[:, None] * inv[None, :]
    emb = np.concatenate([f, f], axis=-1)               # (n, 112)
    return np.cos(emb), np.sin(emb)


def _trig_pad(n):
    c, s = _rope_tables(n)
    cp = np.zeros((HP, n), F32)
    sp = np.zeros((HP, n), F32)
    cp[:HD] = c.T
    sp[:HD] = s.T
    return cp, sp


def _pad_cols(W):
    Wp = np.zeros((W.shape[0], MQ), F32)
    for h in range(NH):
        Wp[:, HP * h:HP * h + HD] = W[:, HD * h:HD * h + HD]
    return Wp


def _pad_rows(W):
    Wp = np.zeros((MQ, W.shape[1]), F32)
    for h in range(NH):
        Wp[HP * h:HP * h + HD, :] = W[HD * h:HD * h + HD, :]
    return Wp


def _pad_vec(b):
    bp = np.zeros(MQ, F32)
    for h in range(NH):
        bp[HP * h:HP * h + HD] = b[HD * h:HD * h + HD]
    return bp


def _lhsT(W, dtype=F16):
    """[Din, M] -> [128, Din//128, M] sbuf layout."""
    Din, M = W.shape
    return np.ascontiguousarray(
        W.reshape(Din // 128, 128, M).transpose(1, 0, 2)).astype(dtype)


def _pk(b):
    """per-partition bias pack: [nm*128] -> [128, nm]"""
    nm = b.shape[0] // 128
    return np.ascontiguousarray(b.reshape(nm, 128).T).astype(F32)


def _shift_T():
    S = np.zeros((HP, HP), F32)
    for i in range(HD // 2):
        S[2 * i, 2 * i + 1] = -1.0
        S[2 * i + 1, 2 * i] = 1.0
    return np.ascontiguousarray(S.T).astype(F16)


def prep_shared(inp, L):
    """Layout transforms shared by all cores (weights etc)."""
    g = {}
    for k, v in inp.items():
        a = np.asarray(v)
        g[k] = a if np.issubdtype(a.dtype, np.integer) else a.astype(F32)
    scale = F32(1.0 / np.sqrt(HD))
    rg = np.tanh(g['gate'])                      # [L]

    wq = np.empty((L, 128, KT, MQ), F16)
    wks = np.empty((L, 128, KT, MQ), F16)
    wka = np.empty((L, 128, KT, MQ), F16)
    wkt = np.empty((L, 128, KT, MQ), F16)
    wvs = np.empty((L, 128, KT, D), F16)
    wva = np.empty((L, 128, KT, D), F16)
    wvt = np.empty((L, 128, KT, D), F16)
    wo = np.empty((L, 128, KTO, D), F16)
    wf = np.empty((L, 128, KT, D), F32)
    bias_pk = np.empty((L, 128, NBP), F32)
    bias_v = np.empty((L, 1, 3, D), F32)

    for l in range(L):
        wq[l] = _lhsT(_pad_cols(g['Wq'][l] * (scale * WSCALE)))
        wks[l] = _lhsT(_pad_cols(g['Wks'][l] * WSCALE))
        wka[l] = _lhsT(_pad_cols(g['Wka'][l] * WSCALE))
        wkt[l] = _lhsT(_pad_cols(g['Wkt'][l] * (rg[l] * WSCALE)))
        wvs[l] = _lhsT(g['Wvs'][l] * WSCALE)
        wva[l] = _lhsT(g['Wva'][l] * WSCALE)
        wvt[l] = _lhsT(g['Wvt'][l] * WSCALE)
        wo[l] = _lhsT(_pad_rows(g['Wo'][l] * WSCALE), F16)
        wf[l] = _lhsT(g['Wf'][l], F32)
        bias_pk[l, :, B_Q:B_Q + 8] = _pk(_pad_vec(g['bq'][l] * scale))
        bias_pk[l, :, B_KS:B_KS + 8] = _pk(_pad_vec(g['bks'][l]))
        bias_pk[l, :, B_KA:B_KA + 8] = _pk(_pad_vec(g['bka'][l]))
        bias_pk[l, :, B_KT:B_KT + 8] = _pk(_pad_vec(g['bkt'][l] * rg[l]))
        bias_pk[l, :, B_O:B_O + 7] = _pk(g['bo'][l])
        bias_pk[l, :, B_F:B_F + 7] = _pk(g['bf'][l])
        bias_pk[l, :, B_G:B_G + 7] = _pk(g['ln_g'][l])
        bias_pk[l, :, B_B:B_B + 7] = _pk(g['ln_b'][l])
        bias_v[l, 0, 0] = g['bvs'][l]
        bias_v[l, 0, 1] = g['bva'][l]
        bias_v[l, 0, 2] = g['bvt'][l]

    trig = np.empty((128, TRIG_W), F32)
    trig[:, O_CQ:O_CQ + T], trig[:, O_SQ:O_SQ + T] = _trig_pad(T)
    trig[:, O_CA:O_CA + NA], trig[:, O_SA:O_SA + NA] = _trig_pad(NA)
    trig[:, O_CT:O_CT + NVIS], trig[:, O_ST:O_ST + NVIS] = _trig_pad(NVIS)

    fin = np.zeros((128, 16), F32)
    fin[:, 0:7] = _pk(g['og'])
    fin[:, 7:14] = _pk(g['ob'])
    fin[:, 14:16] = _pk(g['bout'])

    shared = {
        'wq': wq, 'wks': wks, 'wka': wka, 'wkt': wkt,
        'wvs': wvs, 'wva': wva, 'wvt': wvt, 'wo': wo, 'wf': wf,
        'wout': _lhsT(g['Wout'], F32),
        'bias_pk': bias_pk, 'bias_v': bias_v,
        'trig': trig.astype(F16), 'fin': fin, 'shift_t': _shift_T(),
    }
    return shared, g


def prep_core(g, b, L):
    """Per-core (= per batch element) activations in sbuf layout."""
    mhs = g['multi_layer_hidden_states']
    p = g['proprio'][b] @ g['Wp'] + g['bp']                    # [D]
    ht = np.empty((L, 128, KT, NVIS), F16)
    ha = np.empty((L, 128, KT, NA), F16)
    for l in range(L):
        ht[l] = mhs[b, l, :NVIS, :].T.reshape(KT, 128, NVIS).transpose(1, 0, 2)
        ha_full = np.concatenate([mhs[b, l, NVIS:, :], p[None]], 0).T  # [D, 65]
        ha[l] = ha_full.reshape(KT, 128, NA).transpose(1, 0, 2)
    x0 = g['tok_emb'][np.asarray(g['input_tokens'][b], np.int64)].T   # [D, T]
    x0 = np.ascontiguousarray(x0.reshape(KT, 128, T).transpose(1, 0, 2)).astype(F32)
    return {'ht': ht, 'ha': ha, 'x0': x0}


# ----------------------------------------------------------------------------
# bass program
# ----------------------------------------------------------------------------

def build_program(L, xdbg=False, variant=None):
    import itertools
    _ctr = itertools.count()
    import concourse.tile as tile
    import concourse.mybir as mybir
    from concourse import bacc

    dt = mybir.dt
    AF = mybir.ActivationFunctionType
    OP = mybir.AluOpType
    DS = 1.0 / WSCALE

    nc = bacc.Bacc("TRN2", target_bir_lowering=False, debug=False,
                   num_devices=NCORES, name="ddah")

    def din(name, shape, dtype=dt.float16):
        return nc.dram_tensor(name, shape, dtype, kind="ExternalInput")

    d_wq = din("wq", [L, 128, KT, MQ])
    d_wks = din("wks", [L, 128, KT, MQ])
    d_wka = din("wka", [L, 128, KT, MQ])
    d_wkt = din("wkt", [L, 128, KT, MQ])
    d_wvs = din("wvs", [L, 128, KT, D])
    d_wva = din("wva", [L, 128, KT, D])
    d_wvt = din("wvt", [L, 128, KT, D])
    d_wo = din("wo", [L, 128, KTO, D], dt.float16)
    d_wf = din("wf", [L, 128, KT, D], dt.float32)
    d_wout = din("wout", [128, KT, VOCAB], dt.float32)
    d_bpk = din("bias_pk", [L, 128, NBP], dt.float32)
    d_bv = din("bias_v", [L, 1, 3, D], dt.float32)
    d_trig = din("trig", [128, TRIG_W])
    d_fin = din("fin", [128, 16], dt.float32)
    d_shift = din("shift_t", [128, 128])
    d_ht = din("ht", [L, 128, KT, NVIS])
    d_ha = din("ha", [L, 128, KT, NA])
    d_x0 = din("x0", [128, KT, T], dt.float32)
    d_out = nc.dram_tensor("out", [128, 2, T], dt.float32, kind="ExternalOutput")
    d_xdbg = None
    taps = {}

    def tap(name, tile_, l=0):
        if not xdbg or l != 0 or name in taps:
            return
        dtt = nc.dram_tensor(f"tap_{name}", list(tile_.shape), tile_.dtype,
                             kind="ExternalOutput")
        taps[name] = dtt
        nc.sync.dma_start(dtt[:], tile_)
    if xdbg:
        d_xdbg = nc.dram_tensor("xdbg", [L, 128, KT, T], dt.float32,
                                kind="ExternalOutput")

    MH0, MH1 = 4, 3   # Wo/Wf fp32 m-tile halves: tiles 0-3 (512 cols), 4-6 (384)

    kvb = 1 if variant == "noweightdma" else 2
    with tile.TileContext(nc) as tc, \
         tc.tile_pool(name="singles", bufs=1) as singles, \
         tc.tile_pool(name="wp16", bufs=3) as wp16, \
         tc.tile_pool(name="wp32", bufs=2) as wp32, \
         tc.tile_pool(name="iop", bufs=kvb) as iop, \
         tc.tile_pool(name="kvp", bufs=kvb) as kvp, \
         tc.tile_pool(name="tmp", bufs=kvb) as tmp, \
         tc.tile_pool(name="att", bufs=kvb) as att, \
         tc.tile_pool(name="xp", bufs=kvb * 2) as xp, \
         tc.tile_pool(name="yp", bufs=kvb) as yp, \
         tc.tile_pool(name="st", bufs=2 * kvb) as st, \
         tc.tile_pool(name="bvp", bufs=1) as bvp, \
         tc.tile_pool(name="ps1", bufs=4, space="PSUM") as ps1, \
         tc.tile_pool(name="ps2", bufs=2, space="PSUM") as ps2:

        # ---- constants loaded once ----
        trig = singles.tile([128, TRIG_W], dt.float16, name="trig")
        nc.sync.dma_start(trig, d_trig[:])
        shift = singles.tile([128, 128], dt.float16, name="shift")
        nc.sync.dma_start(shift, d_shift[:])
        fin = singles.tile([128, 16], dt.float32, name="fin")
        nc.sync.dma_start(fin, d_fin[:])
        ones_h = singles.tile([128, 1], dt.float16, name="ones_h")
        nc.vector.memset(ones_h, 1.0)
        ones_f = singles.tile([128, 1], dt.float32, name="ones_f")
        nc.vector.memset(ones_f, 1.0)
        eps_t = singles.tile([1, 1], dt.float32, name="eps_t")
        nc.vector.memset(eps_t, EPS)
        ones_row = singles.tile([1, 128], dt.float32, name="ones_row")
        nc.vector.memset(ones_row, 1.0)
        ds_row = singles.tile([1, 128], dt.float32, name="ds_row")
        nc.vector.memset(ds_row, DS)

        cq, sq = trig[:, O_CQ:O_CQ + T], trig[:, O_SQ:O_SQ + T]
        ca, sa = trig[:, O_CA:O_CA + NA], trig[:, O_SA:O_SA + NA]
        ct, stg = trig[:, O_CT:O_CT + NVIS], trig[:, O_ST:O_ST + NVIS]

        x_sb = xp.tile([128, KT, T], dt.float32, tag="x", name="x_init")
        nc.sync.dma_start(x_sb, d_x0[:])

        hoist = (variant == "noweightdma")
        dma_only = (variant == "dmaonly")
        _hoisted = {}

        def load_w(dram, l, ktiles, m):
            if hoist:
                key = id(dram)
                if key not in _hoisted:
                    w = singles.tile([128, ktiles, m], dt.float16,
                                     name=f"hw{next(_ctr)}")
                    nc.sync.dma_start(w, dram[0])
                    _hoisted[key] = w
                return _hoisted[key]
            w = wp16.tile([128, ktiles, m], dt.float16, tag="w",
                          name=f"t{next(_ctr)}")
            nc.sync.dma_start(w, dram[l])
            return w

        def load_w32(dram, l, ktiles, half):
            cols = slice(0, 512) if half == 0 else slice(512, D)
            n = 512 if half == 0 else D - 512
            if hoist:
                key = id(dram)
                if key not in _hoisted:
                    w = singles.tile([128, ktiles, 512], dt.float32,
                                     name=f"hw{next(_ctr)}")
                    nc.sync.dma_start(w, dram[0][:, :, 0:512])
                    _hoisted[key] = w
                return _hoisted[key][:, :, :n]
            w = wp32.tile([128, ktiles, n], dt.float32, tag="w32",
                          name=f"t{next(_ctr)}")
            nc.sync.dma_start(w, dram[l][:, :, cols])
            return w

        def proj_qk(w_sb, rhs_sb, n, bias_col, cos, sin, bsb):
            """fp16 padded-head projection + rope. Returns [128, NH, n] fp16.

            Engine split: PE matmul -> ACT (scale+bias copy out of PSUM) ->
            PE shift matmul; cos-product on GpSimd, sin-product + add on DVE.
            """
            out = kvp.tile([128, NH, n], dt.float16,
                           tag=f"qk{n}_{bias_col}", name=f"t{next(_ctr)}")
            for h in range(NH):
                ps = ps1.tile([128, 512], dt.float32, tag="ps1",
                              name=f"t{next(_ctr)}")[:, :n]
                for k in range(KT):
                    nc.tensor.matmul(ps, w_sb[:, k, HP * h:HP * h + HP],
                                     rhs_sb[:, k, :],
                                     start=(k == 0), stop=(k == KT - 1))
                ksb = tmp.tile([128, NVIS], dt.float16, tag="rk",
                               name=f"t{next(_ctr)}")[:, :n]
                nc.scalar.activation(ksb, ps, AF.Identity,
                                     bias=bsb[:, bias_col + h:bias_col + h + 1],
                                     scale=DS)
                rps = ps1.tile([128, 512], dt.float32, tag="ps1",
                               name=f"t{next(_ctr)}")[:, :n]
                nc.tensor.matmul(rps, shift, ksb, start=True, stop=True)
                t1 = tmp.tile([128, NVIS], dt.float16, tag="rt1",
                              name=f"t{next(_ctr)}")[:, :n]
                nc.gpsimd.tensor_tensor(t1, ksb, cos, OP.mult)
                t2 = tmp.tile([128, NVIS], dt.float16, tag="rt2",
                              name=f"t{next(_ctr)}")[:, :n]
                nc.vector.tensor_tensor(t2, rps, sin, OP.mult)
                nc.vector.tensor_tensor(out[:, h, :], t1, t2, OP.add)
            return out

        def proj_v(rhs_w, act_sb, mtiles, mlast, tag, bv_row, bvsb):
            """fp16 swapped projection -> [tok, D] layout."""
            out = kvp.tile([128, mtiles, D], dt.float16, tag=f"v{tag}",
                           name=f"t{next(_ctr)}") \
                if mtiles > 1 else \
                kvp.tile([mlast, 1, D], dt.float16, tag=f"v{tag}",
                         name=f"t{next(_ctr)}")
            for m in range(mtiles):
                mw = 128 if m < mtiles - 1 else mlast
                ps = ps2.tile([128, D], dt.float32, tag="ps2",
                              name=f"t{next(_ctr)}")[:mw]
                for sl in (slice(0, 512), slice(512, D)):
                    for k in range(KT):
                        nc.tensor.matmul(
                            ps[:, sl],
                            act_sb[:, k, 128 * m:128 * m + mw],
                            rhs_w[:, k, sl],
                            start=(k == 0), stop=(k == KT - 1))
                nc.vector.scalar_tensor_tensor(
                    out[:mw, m, :], ps, DS, bvsb[:mw, bv_row, :],
                    OP.mult, OP.add)
            return out

        nrep = 1
        if isinstance(variant, tuple) and variant[0] == "rep":
            nrep = variant[1]
        if variant == "empty":
            nrep = 0
        for li in range(L * nrep):
            l = li % L
            if isinstance(variant, str) and variant.startswith("probe_"):
                which = variant[6:]
                if li == 0:
                    sink = singles.tile([128, 1], dt.float32, name="sink")
                if which == "bv":
                    bvsb = bvp.tile([128, 3, D], dt.float32, tag="biasv",
                                    name=f"t{next(_ctr)}")
                    nc.gpsimd.dma_start(bvsb, d_bv[l].to_broadcast((128, 3, D)))
                    nc.vector.tensor_copy(out=sink, in_=bvsb[:, 0, 0:1])
                elif which == "bias":
                    bsb = st.tile([128, NBP], dt.float32, tag="bias",
                                  name=f"t{next(_ctr)}")
                    nc.sync.dma_start(bsb, d_bpk[l])
                    nc.vector.tensor_copy(out=sink, in_=bsb[:, 0:1])
                elif which == "ht":
                    ht_sb = iop.tile([128, KT, NVIS], dt.float16, tag="ht",
                                     name=f"t{next(_ctr)}")
                    nc.sync.dma_start(ht_sb, d_ht[l])
                    nc.vector.tensor_copy(out=sink, in_=ht_sb[:, 0, 0:1])
                elif which == "ha":
                    ha_sb = iop.tile([128, KT, NA], dt.float16, tag="ha",
                                     name=f"t{next(_ctr)}")
                    nc.sync.dma_start(ha_sb, d_ha[l])
                    nc.vector.tensor_copy(out=sink, in_=ha_sb[:, 0, 0:1])
                elif which == "w":
                    w = load_w(d_wq, l, KT, MQ)
                    nc.vector.tensor_copy(out=sink, in_=w[:, 0, 0:1])
                elif which == "w4":
                    for dram in (d_wq, d_wks, d_wka, d_wkt):
                        w = load_w(dram, l, KT, MQ)
                        nc.vector.tensor_copy(out=sink, in_=w[:, 0, 0:1])
                elif which == "wall7":
                    for dram, m in ((d_wq, MQ), (d_wks, MQ), (d_wka, MQ),
                                    (d_wkt, MQ), (d_wvs, D), (d_wva, D),
                                    (d_wvt, D)):
                        w = load_w(dram, l, KT, m)
                        nc.vector.tensor_copy(out=sink, in_=w[:, 0, 0:1])
                continue
            bsb = st.tile([128, NBP], dt.float32, tag="bias",
                          name=f"t{next(_ctr)}")
            nc.sync.dma_start(bsb, d_bpk[l])
            bvsb = bvp.tile([128, 3, D],
                            dt.float16 if hoist else dt.float32, tag="biasv",
                            name=f"t{next(_ctr)}")
            nc.gpsimd.dma_start(bvsb, d_bv[l].to_broadcast((128, 3, D)))
            ht_sb = iop.tile([128, KT, NVIS], dt.float16, tag="ht",
                             name=f"t{next(_ctr)}")
            nc.sync.dma_start(ht_sb, d_ht[l])
            ha_sb = iop.tile([128, KT, NA], dt.float16, tag="ha",
                             name=f"t{next(_ctr)}")
            nc.sync.dma_start(ha_sb, d_ha[l])

            if variant in ("dmabig2", "dmabig4"):
                # split the big transfer across multiple DGE queues
                if li == 0:
                    sink = singles.tile([128, 1], dt.float32, name="sink")
                    WALL = 9556
                    d_wall = nc.dram_tensor("wall", [L, 128, WALL], dt.float16,
                                            kind="ExternalInput")
                w = wp16.tile([128, 9556], dt.float16, tag="wall",
                              name=f"t{next(_ctr)}")
                if variant == "dmabig2":
                    nc.sync.dma_start(w[:, :4778], d_wall[l][:, :4778])
                    nc.gpsimd.dma_start(w[:, 4778:], d_wall[l][:, 4778:])
                else:
                    nc.sync.dma_start(w[:, :3185], d_wall[l][:, :3185])
                    nc.scalar.dma_start(w[:, 3185:6370], d_wall[l][:, 3185:6370])
                    nc.gpsimd.dma_start(w[:, 6370:], d_wall[l][:, 6370:])
                nc.vector.tensor_copy(out=sink, in_=w[:, 0:1])
                continue

            if variant in ("dmabig", "dmabig_sw"):
                # single huge DMA per layer, same total bytes as all weights
                if li == 0:
                    sink = singles.tile([128, 1], dt.float32, name="sink")
                    WALL = 9556  # fp16 elems/partition ~= 19.1MB total
                    d_wall = nc.dram_tensor("wall", [L, 128, WALL], dt.float16,
                                            kind="ExternalInput")
                w = wp16.tile([128, 9556], dt.float16, tag="wall",
                              name=f"t{next(_ctr)}")
                if variant == "dmabig_sw":
                    nc.gpsimd.dma_start(w, d_wall[l])
                else:
                    nc.sync.dma_start(w, d_wall[l])
                nc.vector.tensor_copy(out=sink, in_=w[:, 0:1])
                continue

            if dma_only:
                if li == 0:
                    sink = singles.tile([128, 1], dt.float32, name="sink")
                for dram, kt_, m in ((d_wkt, KT, MQ), (d_wvt, KT, D),
                                     (d_wka, KT, MQ), (d_wva, KT, D),
                                     (d_wq, KT, MQ), (d_wks, KT, MQ),
                                     (d_wvs, KT, D), (d_wo, KTO, D)):
                    w = load_w(dram, l, kt_, m)
                    nc.vector.tensor_copy(out=sink, in_=w[:, 0, 0:1])
                for half in (0, 1):
                    w = load_w32(d_wf, l, KT, half)
                    nc.vector.tensor_copy(out=sink, in_=w[:, 0, 0:1])
                nc.vector.tensor_copy(out=sink, in_=ht_sb[:, 0, 0:1])
                nc.vector.tensor_copy(out=sink, in_=ha_sb[:, 0, 0:1])
                continue

            # fp16 shadow of the fp32 residual stream (on ACT, DVE is busier)
            x16 = xp.tile([128, KT, T], dt.float16, tag="x16",
                          name=f"t{next(_ctr)}")
            nc.scalar.activation(x16, x_sb, AF.Copy)

            # ---- kv path ----
            w = load_w(d_wkt, l, KT, MQ)
            kt_ro = proj_qk(w, ht_sb, NVIS, B_KT, ct, stg, bsb)
            tap("kt_ro", kt_ro, l)
            w = load_w(d_wvt, l, KT, D)
            vt_sb = proj_v(w, ht_sb, 4, 128, "t", 2, bvsb)
            tap("vt", vt_sb, l)
            w = load_w(d_wka, l, KT, MQ)
            ka_ro = proj_qk(w, ha_sb, NA, B_KA, ca, sa, bsb)
            w = load_w(d_wva, l, KT, D)
            va_sb = proj_v(w, ha_sb, 1, NA, "a", 1, bvsb)

            # ---- x projections ----
            w = load_w(d_wq, l, KT, MQ)
            q_ro = proj_qk(w, x16, T, B_Q, cq, sq, bsb)
            tap("q_ro", q_ro, l)
            w = load_w(d_wks, l, KT, MQ)
            ks_ro = proj_qk(w, x16, T, B_KS, cq, sq, bsb)
            w = load_w(d_wvs, l, KT, D)
            vs_sb = proj_v(w, x16, 1, T, "s", 0, bvsb)

            # ---- attention (scores transposed: [keys, queries]) ----
            NT = NH * T  # 448
            ex_s = att.tile([T, NT], dt.float16, tag="exs",
                            name=f"t{next(_ctr)}")
            ex_a = att.tile([NA, NT], dt.float16, tag="exa",
                            name=f"t{next(_ctr)}")
            ex_t = att.tile([128, 4, NT], dt.float16, tag="ext",
                            name=f"t{next(_ctr)}")

            ps = ps1.tile([T, 512], dt.float32, tag="ps1",
                          name=f"t{next(_ctr)}")[:, :NT]
            for h in range(NH):
                nc.tensor.matmul(ps[:, T * h:T * h + T], ks_ro[:, h, :],
                                 q_ro[:, h, :], start=True, stop=True)
            nc.scalar.activation(ex_s, ps, AF.Exp)
            tap("ex_s", ex_s, l)
            ps = ps1.tile([NA, 512], dt.float32, tag="ps1",
                          name=f"t{next(_ctr)}")[:, :NT]
            for h in range(NH):
                nc.tensor.matmul(ps[:, T * h:T * h + T], ka_ro[:, h, :],
                                 q_ro[:, h, :], start=True, stop=True)
            nc.scalar.activation(ex_a, ps, AF.Exp)
            for m in range(4):
                ps = ps1.tile([128, 512], dt.float32, tag="ps1",
                              name=f"t{next(_ctr)}")[:, :NT]
                for h in range(NH):
                    nc.tensor.matmul(ps[:, T * h:T * h + T],
                                     kt_ro[:, h, 128 * m:128 * m + 128],
                                     q_ro[:, h, :], start=True, stop=True)
                nc.scalar.activation(ex_t[:, m, :], ps, AF.Exp)

            lps = ps1.tile([1, 512], dt.float32, tag="ps1",
                           name=f"t{next(_ctr)}")[:, :NT]
            nc.tensor.matmul(lps, ones_h[:T], ex_s, start=True, stop=False)
            nc.tensor.matmul(lps, ones_h[:NA], ex_a, start=False, stop=False)
            for m in range(4):
                nc.tensor.matmul(lps, ones_h, ex_t[:, m, :],
                                 start=False, stop=(m == 3))
            linv = st.tile([1, NT], dt.float32, tag="linv",
                           name=f"t{next(_ctr)}")
            nc.vector.reciprocal(linv, lps)
            tap("linv", linv, l)
            # linv_b carries DS so that o16 = o * DS; wo carries WSCALE.
            lrep_ps = ps1.tile([128, 512], dt.float32, tag="ps1",
                               name=f"t{next(_ctr)}")[:, :NT]
            nc.tensor.matmul(lrep_ps, ds_row, linv, start=True, stop=True)
            linv_b = att.tile([128, NT], dt.float32, tag="linvb",
                              name=f"t{next(_ctr)}")
            nc.vector.tensor_copy(out=linv_b, in_=lrep_ps)

            o16 = att.tile([128, NH, T], dt.float16, tag="opad",
                           name=f"t{next(_ctr)}")
            nc.vector.memset(o16[96:128], 0.0)
            for h in range(NH):
                hs = slice(HD * h, HD * h + HD)
                ops = ps1.tile([HD, 512], dt.float32, tag="ps1",
                               name=f"t{next(_ctr)}")[:, :T]
                nc.tensor.matmul(ops, vs_sb[:T, 0, hs], ex_s[:, T * h:T * h + T],
                                 start=True, stop=False)
                nc.tensor.matmul(ops, va_sb[:NA, 0, hs], ex_a[:, T * h:T * h + T],
                                 start=False, stop=False)
                for m in range(4):
                    nc.tensor.matmul(ops, vt_sb[:, m, hs],
                                     ex_t[:, m, T * h:T * h + T],
                                     start=False, stop=(m == 3))
                nc.vector.tensor_tensor(
                    o16[:HD, h, :], ops, linv_b[:HD, T * h:T * h + T], OP.mult)
            tap("o_pad", o16, l)

            # ---- Wo (fp16, scale pre-folded) + residual ----
            y_sb = yp.tile([128, KT, T], dt.float32, tag="y",
                           name=f"t{next(_ctr)}")
            w = load_w(d_wo, l, KTO, D)
            for mo in range(KT):
                ps = ps1.tile([128, 512], dt.float32, tag="ps1",
                              name=f"t{next(_ctr)}")[:, :T]
                for k in range(KTO):
                    nc.tensor.matmul(ps, w[:, k, 128 * mo:128 * mo + 128],
                                     o16[:, k, :],
                                     start=(k == 0), stop=(k == KTO - 1))
                nc.vector.scalar_tensor_tensor(
                    y_sb[:, mo, :], ps, bsb[:, B_O + mo:B_O + mo + 1],
                    x_sb[:, mo, :], OP.add, OP.add)
            tap("y", y_sb, l)

            # ---- layernorm (fp32) ----
            mps = ps1.tile([1, 512], dt.float32, tag="ps1",
                           name=f"t{next(_ctr)}")[:, :T]
            for k in range(KT):
                nc.tensor.matmul(mps, ones_f, y_sb[:, k, :],
                                 start=(k == 0), stop=(k == KT - 1))
            ysq = yp.tile([128, KT, T], dt.float32, tag="ysq",
                          name=f"t{next(_ctr)}")
            nc.scalar.activation(ysq, y_sb, AF.Square)
            sps = ps1.tile([1, 512], dt.float32, tag="ps1",
                           name=f"t{next(_ctr)}")[:, :T]
            for k in range(KT):
                nc.tensor.matmul(sps, ones_f, ysq[:, k, :],
                                 start=(k == 0), stop=(k == KT - 1))
            mean = st.tile([1, T], dt.float32, tag="mean",
                           name=f"t{next(_ctr)}")
            nc.vector.tensor_scalar_mul(mean, mps, 1.0 / D)
            msq = st.tile([1, T], dt.float32, tag="msq", name=f"t{next(_ctr)}")
            nc.vector.tensor_tensor(msq, mean, mean, OP.mult)
            var = st.tile([1, T], dt.float32, tag="var", name=f"t{next(_ctr)}")
            nc.vector.scalar_tensor_tensor(var, sps, 1.0 / D, msq,
                                           OP.mult, OP.subtract)
            rc = st.tile([1, 2 * T], dt.float32, tag="rc", name=f"t{next(_ctr)}")
            nc.scalar.activation(rc[:, :T], var, AF.Sqrt, bias=eps_t)
            nc.vector.reciprocal(rc[:, :T], rc[:, :T])
            nc.vector.tensor_tensor(rc[:, T:], mean, rc[:, :T], OP.mult)
            rrep_ps = ps1.tile([128, 512], dt.float32, tag="ps1",
                               name=f"t{next(_ctr)}")[:, :2 * T]
            nc.tensor.matmul(rrep_ps, ones_row, rc, start=True, stop=True)
            rc_b = st.tile([128, 2 * T], dt.float32, tag="rcb",
                           name=f"t{next(_ctr)}")
            nc.vector.tensor_copy(out=rc_b, in_=rrep_ps)
            lnf = yp.tile([128, KT, T], dt.float32, tag="lnf",
                          name=f"t{next(_ctr)}")
            for k in range(KT):
                t1 = tmp.tile([128, T], dt.float32, tag="lt1",
                              name=f"t{next(_ctr)}")
                nc.vector.tensor_tensor(t1, y_sb[:, k, :], rc_b[:, :T], OP.mult)
                t2 = tmp.tile([128, T], dt.float32, tag="lt2",
                              name=f"t{next(_ctr)}")
                nc.vector.tensor_tensor(t2, t1, rc_b[:, T:], OP.subtract)
                nc.vector.tensor_scalar(lnf[:, k, :], t2,
                                        bsb[:, B_G + k:B_G + k + 1],
                                        bsb[:, B_B + k:B_B + k + 1],
                                        OP.mult, OP.add)
            tap("ln", lnf, l)

            # ---- Wf (fp32) + relu -> new x ----
            x_new = xp.tile([128, KT, T], dt.float32, tag="x",
                            name=f"t{next(_ctr)}")
            for half, mos in ((0, range(4)), (1, range(4, KT))):
                w32 = load_w32(d_wf, l, KT, half)
                for mo in mos:
                    mc = mo * 128 - half * 512
                    ps = ps1.tile([128, 512], dt.float32, tag="ps1",
                                  name=f"t{next(_ctr)}")[:, :T]
                    for k in range(KT):
                        nc.tensor.matmul(ps, w32[:, k, mc:mc + 128],
                                         lnf[:, k, :],
                                         start=(k == 0), stop=(k == KT - 1))
                    nc.scalar.activation(x_new[:, mo, :], ps, AF.Relu,
                                         bias=bsb[:, B_F + mo:B_F + mo + 1])
            x_sb = x_new
            if d_xdbg is not None and li == l:
                nc.sync.dma_start(d_xdbg[l], x_sb)

        # ---- final layernorm + Wout (fp32) ----
        mps = ps1.tile([1, 512], dt.float32, tag="ps1",
                       name=f"t{next(_ctr)}")[:, :T]
        for k in range(KT):
            nc.tensor.matmul(mps, ones_f, x_sb[:, k, :],
                             start=(k == 0), stop=(k == KT - 1))
        ysq = yp.tile([128, KT, T], dt.float32, tag="ysq", name=f"t{next(_ctr)}")
        nc.scalar.activation(ysq, x_sb, AF.Square)
        sps = ps1.tile([1, 512], dt.float32, tag="ps1",
                       name=f"t{next(_ctr)}")[:, :T]
        for k in range(KT):
            nc.tensor.matmul(sps, ones_f, ysq[:, k, :],
                             start=(k == 0), stop=(k == KT - 1))
        mean = st.tile([1, T], dt.float32, tag="mean", name=f"t{next(_ctr)}")
        nc.vector.tensor_scalar_mul(mean, mps, 1.0 / D)
        msq = st.tile([1, T], dt.float32, tag="msq", name=f"t{next(_ctr)}")
        nc.vector.tensor_tensor(msq, mean, mean, OP.mult)
        var = st.tile([1, T], dt.float32, tag="var", name=f"t{next(_ctr)}")
        nc.vector.scalar_tensor_tensor(var, sps, 1.0 / D, msq,
                                       OP.mult, OP.subtract)
        rc = st.tile([1, 2 * T], dt.float32, tag="rc", name=f"t{next(_ctr)}")
        nc.scalar.activation(rc[:, :T], var, AF.Sqrt, bias=eps_t)
        nc.vector.reciprocal(rc[:, :T], rc[:, :T])
        nc.vector.tensor_tensor(rc[:, T:], mean, rc[:, :T], OP.mult)
        rrep_ps = ps1.tile([128, 512], dt.float32, tag="ps1",
                           name=f"t{next(_ctr)}")[:, :2 * T]
        nc.tensor.matmul(rrep_ps, ones_row, rc, start=True, stop=True)
        rc_b = st.tile([128, 2 * T], dt.float32, tag="rcb", name=f"t{next(_ctr)}")
        nc.vector.tensor_copy(out=rc_b, in_=rrep_ps)
        lnf = yp.tile([128, KT, T], dt.float32, tag="lnf", name=f"t{next(_ctr)}")
        for k in range(KT):
            t1 = tmp.tile([128, T], dt.float32, tag="lt1", name=f"t{next(_ctr)}")
            nc.vector.tensor_tensor(t1, x_sb[:, k, :], rc_b[:, :T], OP.mult)
            t2 = tmp.tile([128, T], dt.float32, tag="lt2", name=f"t{next(_ctr)}")
            nc.vector.tensor_tensor(t2, t1, rc_b[:, T:], OP.subtract)
            nc.vector.tensor_scalar(lnf[:, k, :], t2,
                                    fin[:, k:k + 1], fin[:, 7 + k:8 + k],
                                    OP.mult, OP.add)
        woutsb = wp32.tile([128, KT, VOCAB], dt.float32, tag="w32",
                           name="woutsb")
        nc.sync.dma_start(woutsb, d_wout[:])
        out_sb = yp.tile([128, 2, T], dt.float32, tag="outsb",
                         name=f"t{next(_ctr)}")
        for mo in range(2):
            ps = ps1.tile([128, 512], dt.float32, tag="ps1",
                          name=f"t{next(_ctr)}")[:, :T]
            for k in range(KT):
                nc.tensor.matmul(ps, woutsb[:, k, 128 * mo:128 * mo + 128],
                                 lnf[:, k, :],
                                 start=(k == 0), stop=(k == KT - 1))
            nc.vector.tensor_scalar_add(out_sb[:, mo, :], ps,
                                        fin[:, 14 + mo:15 + mo])
        nc.sync.dma_start(d_out[:], out_sb)

    nc.compile()
    return nc


_PROG_CACHE = {}


def _get_program(L, xdbg=False, variant=None):
    key = (L, xdbg, variant)
    if key not in _PROG_CACHE:
        _PROG_CACHE[key] = build_program(L, xdbg, variant)
    return _PROG_CACHE[key]


def run(inputs, L=L_FULL, xdbg=False):
    from concourse.bass_utils import run_bass_kernel_spmd
    nc = _get_program(L, xdbg)
    shared, g = prep_shared(inputs, L)
    in_maps = []
    for b in range(NCORES):
        m = dict(shared)
        m.update(prep_core(g, b, L))
        in_maps.append(m)
    res = run_bass_kernel_spmd(nc, in_maps, core_ids=list(range(NCORES)))
    outs = []
    for r in res.results:
        o = r["out"]                                    # [128, 2, T]
        outs.append(np.ascontiguousarray(o.transpose(2, 1, 0)).reshape(T, VOCAB))
    full = np.stack(outs).astype(F32)                   # [B, T, VOCAB]
    if xdbg:
        return full, res.results
    return full


def kernel(**inputs) -> np.ndarray:
    return run(inputs, L=L_FULL)



# revision 40
# speedup vs baseline: 1.1915x; 1.0112x over previous
"""Trainium2 Bass kernel for nn_DiscreteDiffusionActionHead.

Strategy: pure data-parallel over batch (B=8 -> 1 element per NeuronCore,
no collectives). All activations held in [dim(partitions), token(free)]
layout. bf16 matmul inputs with fp32 PSUM accumulation.

Host-side (free, not on HW critical path):
  - token-embedding gather, proprio projection, rope cos/sin tables
  - fold attention scale into Wq/bq, tanh(gate) into Wkt/bkt
  - pad q/k projection output dims per head (112 -> 128) so each head owns
    one partition tile; pad Wo input dim to match
  - rot_half expressed as a [128,128] shift matrix applied on the PE
  - downcast + pre-tile all weights/hidden-states into exact SBUF layouts
"""
import numpy as np
import ml_dtypes

BF16 = ml_dtypes.bfloat16
F16 = np.float16
F32 = np.float32
WSCALE = 256.0

L_FULL = 24
D = 896
NH = 8
HD = 112
HP = 128
MQ = NH * HP            # 1024
KT = D // 128           # 7
KTO = MQ // 128         # 8
T = 56
NVIS = 512
NADP = 64
NA = NADP + 1           # 65
VOCAB = 256
PD = 8
EPS = 1e-5
NCORES = 8

# trig pack offsets (free-dim columns)
TRIG_W = 2 * (T + NA + NVIS)  # 1266
O_CQ, O_SQ = 0, T
O_CA, O_SA = 2 * T, 2 * T + NA
O_CT, O_ST = 2 * T + 2 * NA, 2 * T + 2 * NA + NVIS

# bias pack offsets (per-partition [128, 60])
B_Q, B_KS, B_KA, B_KT = 0, 8, 16, 24
B_O, B_F, B_G, B_B = 32, 39, 46, 53
NBP = 60


# ----------------------------------------------------------------------------
# host-side layout helpers
# ----------------------------------------------------------------------------

def _rope_tables(n):
    inv = 1.0 / (10000.0 ** (np.arange(0, HD, 2, dtype=F32) / HD))
    f = np.arange(n, dtype=F32)[:, None] * inv[None, :]
    emb = np.concatenate([f, f], axis=-1)               # (n, 112)
    return np.cos(emb), np.sin(emb)


def _trig_pad(n):
    c, s = _rope_tables(n)
    cp = np.zeros((HP, n), F32)
    sp = np.zeros((HP, n), F32)
    cp[:HD] = c.T
    sp[:HD] = s.T
    return cp, sp


def _pad_cols(W):
    Wp = np.zeros((W.shape[0], MQ), F32)
    for h in range(NH):
        Wp[:, HP * h:HP * h + HD] = W[:, HD * h:HD * h + HD]
    return Wp


def _pad_rows(W):
    Wp = np.zeros((MQ, W.shape[1]), F32)
    for h in range(NH):
        Wp[HP * h:HP * h + HD, :] = W[HD * h:HD * h + HD, :]
    return Wp


def _pad_vec(b):
    bp = np.zeros(MQ, F32)
    for h in range(NH):
        bp[HP * h:HP * h + HD] = b[HD * h:HD * h + HD]
    return bp


def _lhsT(W, dtype=F16):
    """[Din, M] -> [128, Din//128, M] sbuf layout."""
    Din, M = W.shape
    return np.ascontiguousarray(
        W.reshape(Din // 128, 128, M).transpose(1, 0, 2)).astype(dtype)


def _pk(b):
    """per-partition bias pack: [nm*128] -> [128, nm]"""
    nm = b.shape[0] // 128
    return np.ascontiguousarray(b.reshape(nm, 128).T).astype(F32)


def _shift_T():
    S = np.zeros((HP, HP), F32)
    for i in range(HD // 2):
        S[2 * i, 2 * i + 1] = -1.0
        S[2 * i + 1, 2 * i] = 1.0
    return np.ascontiguousarray(S.T).astype(F16)


def prep_shared(inp, L):
    """Layout transforms shared by all cores (weights etc)."""
    g = {}
    for k, v in inp.items():
        a = np.asarray(v)
        g[k] = a if np.issubdtype(a.dtype, np.integer) else a.astype(F32)
    scale = F32(1.0 / np.sqrt(HD))
    rg = np.tanh(g['gate'])                      # [L]

    wq = np.empty((L, 128, KT, MQ), F16)
    wks = np.empty((L, 128, KT, MQ), F16)
    wka = np.empty((L, 128, KT, MQ), F16)
    wkt = np.empty((L, 128, KT, MQ), F16)
    wvs = np.empty((L, 128, KT, D), F16)
    wva = np.empty((L, 128, KT, D), F16)
    wvt = np.empty((L, 128, KT, D), F16)
    wo = np.empty((L, 128, KTO, D), F16)
    wf = np.empty((L, 128, KT, D), F32)
    bias_pk = np.empty((L, 128, NBP), F32)
    bias_v = np.empty((L, 1, 3, D), F16)

    for l in range(L):
        wq[l] = _lhsT(_pad_cols(g['Wq'][l] * (scale * WSCALE)))
        wks[l] = _lhsT(_pad_cols(g['Wks'][l] * WSCALE))
        wka[l] = _lhsT(_pad_cols(g['Wka'][l] * WSCALE))
        wkt[l] = _lhsT(_pad_cols(g['Wkt'][l] * (rg[l] * WSCALE)))
        wvs[l] = _lhsT(g['Wvs'][l] * WSCALE)
        wva[l] = _lhsT(g['Wva'][l] * WSCALE)
        wvt[l] = _lhsT(g['Wvt'][l] * WSCALE)
        wo[l] = _lhsT(_pad_rows(g['Wo'][l] * WSCALE), F16)
        wf[l] = _lhsT(g['Wf'][l], F32)
        bias_pk[l, :, B_Q:B_Q + 8] = _pk(_pad_vec(g['bq'][l] * scale))
        bias_pk[l, :, B_KS:B_KS + 8] = _pk(_pad_vec(g['bks'][l]))
        bias_pk[l, :, B_KA:B_KA + 8] = _pk(_pad_vec(g['bka'][l]))
        bias_pk[l, :, B_KT:B_KT + 8] = _pk(_pad_vec(g['bkt'][l] * rg[l]))
        bias_pk[l, :, B_O:B_O + 7] = _pk(g['bo'][l])
        bias_pk[l, :, B_F:B_F + 7] = _pk(g['bf'][l])
        bias_pk[l, :, B_G:B_G + 7] = _pk(g['ln_g'][l])
        bias_pk[l, :, B_B:B_B + 7] = _pk(g['ln_b'][l])
        bias_v[l, 0, 0] = g['bvs'][l]
        bias_v[l, 0, 1] = g['bva'][l]
        bias_v[l, 0, 2] = g['bvt'][l]

    trig = np.empty((128, TRIG_W), F32)
    trig[:, O_CQ:O_CQ + T], trig[:, O_SQ:O_SQ + T] = _trig_pad(T)
    trig[:, O_CA:O_CA + NA], trig[:, O_SA:O_SA + NA] = _trig_pad(NA)
    trig[:, O_CT:O_CT + NVIS], trig[:, O_ST:O_ST + NVIS] = _trig_pad(NVIS)

    fin = np.zeros((128, 16), F32)
    fin[:, 0:7] = _pk(g['og'])
    fin[:, 7:14] = _pk(g['ob'])
    fin[:, 14:16] = _pk(g['bout'])

    shared = {
        'wq': wq, 'wks': wks, 'wka': wka, 'wkt': wkt,
        'wvs': wvs, 'wva': wva, 'wvt': wvt, 'wo': wo, 'wf': wf,
        'wout': _lhsT(g['Wout'], F32),
        'bias_pk': bias_pk, 'bias_v': bias_v,
        'trig': trig.astype(F16), 'fin': fin, 'shift_t': _shift_T(),
    }
    return shared, g


def prep_core(g, b, L):
    """Per-core (= per batch element) activations in sbuf layout."""
    mhs = g['multi_layer_hidden_states']
    p = g['proprio'][b] @ g['Wp'] + g['bp']                    # [D]
    ht = np.empty((L, 128, KT, NVIS), F16)
    ha = np.empty((L, 128, KT, NA), F16)
    for l in range(L):
        ht[l] = mhs[b, l, :NVIS, :].T.reshape(KT, 128, NVIS).transpose(1, 0, 2)
        ha_full = np.concatenate([mhs[b, l, NVIS:, :], p[None]], 0).T  # [D, 65]
        ha[l] = ha_full.reshape(KT, 128, NA).transpose(1, 0, 2)
    x0 = g['tok_emb'][np.asarray(g['input_tokens'][b], np.int64)].T   # [D, T]
    x0 = np.ascontiguousarray(x0.reshape(KT, 128, T).transpose(1, 0, 2)).astype(F32)
    return {'ht': ht, 'ha': ha, 'x0': x0}


# ----------------------------------------------------------------------------
# bass program
# ----------------------------------------------------------------------------

def build_program(L, xdbg=False, variant=None):
    import itertools
    _ctr = itertools.count()
    import concourse.tile as tile
    import concourse.mybir as mybir
    from concourse import bacc

    dt = mybir.dt
    AF = mybir.ActivationFunctionType
    OP = mybir.AluOpType
    DS = 1.0 / WSCALE

    nc = bacc.Bacc("TRN2", target_bir_lowering=False, debug=False,
                   num_devices=NCORES, name="ddah")

    def din(name, shape, dtype=dt.float16):
        return nc.dram_tensor(name, shape, dtype, kind="ExternalInput")

    d_wq = din("wq", [L, 128, KT, MQ])
    d_wks = din("wks", [L, 128, KT, MQ])
    d_wka = din("wka", [L, 128, KT, MQ])
    d_wkt = din("wkt", [L, 128, KT, MQ])
    d_wvs = din("wvs", [L, 128, KT, D])
    d_wva = din("wva", [L, 128, KT, D])
    d_wvt = din("wvt", [L, 128, KT, D])
    d_wo = din("wo", [L, 128, KTO, D], dt.float16)
    d_wf = din("wf", [L, 128, KT, D], dt.float32)
    d_wout = din("wout", [128, KT, VOCAB], dt.float32)
    d_bpk = din("bias_pk", [L, 128, NBP], dt.float32)
    d_bv = din("bias_v", [L, 1, 3, D], dt.float16)
    d_trig = din("trig", [128, TRIG_W])
    d_fin = din("fin", [128, 16], dt.float32)
    d_shift = din("shift_t", [128, 128])
    d_ht = din("ht", [L, 128, KT, NVIS])
    d_ha = din("ha", [L, 128, KT, NA])
    d_x0 = din("x0", [128, KT, T], dt.float32)
    d_out = nc.dram_tensor("out", [128, 2, T], dt.float32, kind="ExternalOutput")
    d_xdbg = None
    taps = {}

    def tap(name, tile_, l=0):
        if not xdbg or l != 0 or name in taps:
            return
        dtt = nc.dram_tensor(f"tap_{name}", list(tile_.shape), tile_.dtype,
                             kind="ExternalOutput")
        taps[name] = dtt
        nc.sync.dma_start(dtt[:], tile_)
    if xdbg:
        d_xdbg = nc.dram_tensor("xdbg", [L, 128, KT, T], dt.float32,
                                kind="ExternalOutput")

    MH0, MH1 = 4, 3   # Wo/Wf fp32 m-tile halves: tiles 0-3 (512 cols), 4-6 (384)

    kvb = 1 if variant == "noweightdma" else 2
    wb = 3 if variant == "wb3" else 4
    with tile.TileContext(nc) as tc, \
         tc.tile_pool(name="singles", bufs=1) as singles, \
         tc.tile_pool(name="wp16", bufs=wb) as wp16, \
         tc.tile_pool(name="wp32", bufs=2) as wp32, \
         tc.tile_pool(name="iop", bufs=kvb) as iop, \
         tc.tile_pool(name="kvp", bufs=kvb) as kvp, \
         tc.tile_pool(name="tmp", bufs=kvb) as tmp, \
         tc.tile_pool(name="att", bufs=kvb) as att, \
         tc.tile_pool(name="xp", bufs=kvb * 2) as xp, \
         tc.tile_pool(name="yp", bufs=kvb) as yp, \
         tc.tile_pool(name="st", bufs=2 * kvb) as st, \
         tc.tile_pool(name="bvp", bufs=1) as bvp, \
         tc.tile_pool(name="ps1", bufs=4, space="PSUM") as ps1, \
         tc.tile_pool(name="ps2", bufs=2, space="PSUM") as ps2:

        # ---- constants loaded once ----
        trig = singles.tile([128, TRIG_W], dt.float16, name="trig")
        nc.sync.dma_start(trig, d_trig[:])
        shift = singles.tile([128, 128], dt.float16, name="shift")
        nc.sync.dma_start(shift, d_shift[:])
        fin = singles.tile([128, 16], dt.float32, name="fin")
        nc.sync.dma_start(fin, d_fin[:])
        ones_h = singles.tile([128, 1], dt.float16, name="ones_h")
        nc.vector.memset(ones_h, 1.0)
        ones_f = singles.tile([128, 1], dt.float32, name="ones_f")
        nc.vector.memset(ones_f, 1.0)
        eps_t = singles.tile([1, 1], dt.float32, name="eps_t")
        nc.vector.memset(eps_t, EPS)
        ones_row = singles.tile([1, 128], dt.float32, name="ones_row")
        nc.vector.memset(ones_row, 1.0)
        ds_row = singles.tile([1, 128], dt.float32, name="ds_row")
        nc.vector.memset(ds_row, DS)

        cq, sq = trig[:, O_CQ:O_CQ + T], trig[:, O_SQ:O_SQ + T]
        ca, sa = trig[:, O_CA:O_CA + NA], trig[:, O_SA:O_SA + NA]
        ct, stg = trig[:, O_CT:O_CT + NVIS], trig[:, O_ST:O_ST + NVIS]

        x_sb = xp.tile([128, KT, T], dt.float32, tag="x", name="x_init")
        nc.sync.dma_start(x_sb, d_x0[:])

        hoist = (variant == "noweightdma")
        dma_only = (variant == "dmaonly")
        _hoisted = {}

        def load_w(dram, l, ktiles, m):
            if hoist:
                key = id(dram)
                if key not in _hoisted:
                    w = singles.tile([128, ktiles, m], dt.float16,
                                     name=f"hw{next(_ctr)}")
                    nc.sync.dma_start(w, dram[0])
                    _hoisted[key] = w
                return _hoisted[key]
            w = wp16.tile([128, ktiles, m], dt.float16, tag="w",
                          name=f"t{next(_ctr)}")
            nc.sync.dma_start(w, dram[l])
            return w

        def load_w32(dram, l, ktiles, half):
            cols = slice(0, 512) if half == 0 else slice(512, D)
            n = 512 if half == 0 else D - 512
            if hoist:
                key = id(dram)
                if key not in _hoisted:
                    w = singles.tile([128, ktiles, 512], dt.float32,
                                     name=f"hw{next(_ctr)}")
                    nc.sync.dma_start(w, dram[0][:, :, 0:512])
                    _hoisted[key] = w
                return _hoisted[key][:, :, :n]
            w = wp32.tile([128, ktiles, n], dt.float32, tag="w32",
                          name=f"t{next(_ctr)}")
            nc.sync.dma_start(w, dram[l][:, :, cols])
            return w

        def proj_qk(w_sb, rhs_sb, n, bias_col, cos, sin, bsb,
                    out=None, col0=0):
            """fp16 padded-head projection + rope. Returns [128, NH, n] fp16.

            Engine split: PE matmul -> ACT (scale+bias copy out of PSUM) ->
            PE shift matmul; cos-product on GpSimd, sin-product + add on DVE.
            """
            if out is None:
                out = kvp.tile([128, NH, n], dt.float16,
                               tag=f"qk{n}_{bias_col}", name=f"t{next(_ctr)}")
            full, out = out, out[:, :, col0:col0 + n]
            for h in range(NH):
                ps = ps1.tile([128, 512], dt.float32, tag="ps1",
                              name=f"t{next(_ctr)}")[:, :n]
                for k in range(KT):
                    nc.tensor.matmul(ps, w_sb[:, k, HP * h:HP * h + HP],
                                     rhs_sb[:, k, :],
                                     start=(k == 0), stop=(k == KT - 1))
                ksb = tmp.tile([128, NVIS], dt.float16, tag="rk",
                               name=f"t{next(_ctr)}")[:, :n]
                nc.scalar.activation(ksb, ps, AF.Identity,
                                     bias=bsb[:, bias_col + h:bias_col + h + 1],
                                     scale=DS)
                rps = ps1.tile([128, 512], dt.float32, tag="ps1",
                               name=f"t{next(_ctr)}")[:, :n]
                nc.tensor.matmul(rps, shift, ksb, start=True, stop=True)
                t1 = tmp.tile([128, NVIS], dt.float16, tag="rt1",
                              name=f"t{next(_ctr)}")[:, :n]
                nc.gpsimd.tensor_tensor(t1, ksb, cos, OP.mult)
                t2 = tmp.tile([128, NVIS], dt.float16, tag="rt2",
                              name=f"t{next(_ctr)}")[:, :n]
                nc.vector.tensor_tensor(t2, rps, sin, OP.mult)
                if variant == "ropedve":
                    nc.vector.tensor_tensor(out[:, h, :], t1, t2, OP.add)
                else:
                    nc.gpsimd.tensor_tensor(out[:, h, :], t1, t2, OP.add)
            return full

        def proj_v(rhs_w, act_sb, mtiles, mlast, tag, bv_row, bvsb,
                   out=None, row0=0):
            """fp16 swapped projection -> [tok, D] layout."""
            if out is None:
                out = kvp.tile([128, mtiles, D], dt.float16, tag=f"v{tag}",
                               name=f"t{next(_ctr)}") \
                    if mtiles > 1 else \
                    kvp.tile([row0 + mlast, 1, D], dt.float16, tag=f"v{tag}",
                             name=f"t{next(_ctr)}")
            for m in range(mtiles):
                mw = 128 if m < mtiles - 1 else mlast
                r0 = row0 if mtiles == 1 else 0
                ps = ps2.tile([128, D], dt.float32, tag="ps2",
                              name=f"t{next(_ctr)}")[r0:r0 + mw]
                for sl in (slice(0, 512), slice(512, D)):
                    for k in range(KT):
                        nc.tensor.matmul(
                            ps[:, sl],
                            act_sb[:, k, 128 * m:128 * m + mw],
                            rhs_w[:, k, sl],
                            start=(k == 0), stop=(k == KT - 1))
                nc.vector.scalar_tensor_tensor(
                    out[r0:r0 + mw, m, :], ps, DS,
                    bvsb[r0:r0 + mw, bv_row, :],
                    OP.mult, OP.add)
            return out

        nrep = 1
        if isinstance(variant, tuple) and variant[0] == "rep":
            nrep = variant[1]
        if variant == "empty":
            nrep = 0
        for li in range(L * nrep):
            l = li % L
            if isinstance(variant, str) and variant.startswith("probe_"):
                which = variant[6:]
                if li == 0:
                    sink = singles.tile([128, 1], dt.float32, name="sink")
                if which == "bv":
                    bvsb = bvp.tile([128, 3, D], dt.float16, tag="biasv",
                                    name=f"t{next(_ctr)}")
                    nc.gpsimd.dma_start(bvsb, d_bv[l].to_broadcast((128, 3, D)))
                    nc.vector.tensor_copy(out=sink, in_=bvsb[:, 0, 0:1])
                elif which == "bias":
                    bsb = st.tile([128, NBP], dt.float32, tag="bias",
                                  name=f"t{next(_ctr)}")
                    nc.sync.dma_start(bsb, d_bpk[l])
                    nc.vector.tensor_copy(out=sink, in_=bsb[:, 0:1])
                elif which == "ht":
                    ht_sb = iop.tile([128, KT, NVIS], dt.float16, tag="ht",
                                     name=f"t{next(_ctr)}")
                    nc.sync.dma_start(ht_sb, d_ht[l])
                    nc.vector.tensor_copy(out=sink, in_=ht_sb[:, 0, 0:1])
                elif which == "ha":
                    ha_sb = iop.tile([128, KT, NA], dt.float16, tag="ha",
                                     name=f"t{next(_ctr)}")
                    nc.sync.dma_start(ha_sb, d_ha[l])
                    nc.vector.tensor_copy(out=sink, in_=ha_sb[:, 0, 0:1])
                elif which == "w":
                    w = load_w(d_wq, l, KT, MQ)
                    nc.vector.tensor_copy(out=sink, in_=w[:, 0, 0:1])
                elif which == "w4":
                    for dram in (d_wq, d_wks, d_wka, d_wkt):
                        w = load_w(dram, l, KT, MQ)
                        nc.vector.tensor_copy(out=sink, in_=w[:, 0, 0:1])
                elif which == "wall7":
                    for dram, m in ((d_wq, MQ), (d_wks, MQ), (d_wka, MQ),
                                    (d_wkt, MQ), (d_wvs, D), (d_wva, D),
                                    (d_wvt, D)):
                        w = load_w(dram, l, KT, m)
                        nc.vector.tensor_copy(out=sink, in_=w[:, 0, 0:1])
                continue
            bsb = st.tile([128, NBP], dt.float32, tag="bias",
                          name=f"t{next(_ctr)}")
            nc.sync.dma_start(bsb, d_bpk[l])
            bvsb = bvp.tile([128, 3, D], dt.float16, tag="biasv",
                            name=f"t{next(_ctr)}")
            nc.gpsimd.dma_start(bvsb, d_bv[l].to_broadcast((128, 3, D)))
            ht_sb = iop.tile([128, KT, NVIS], dt.float16, tag="ht",
                             name=f"t{next(_ctr)}")
            nc.sync.dma_start(ht_sb, d_ht[l])
            ha_sb = iop.tile([128, KT, NA], dt.float16, tag="ha",
                             name=f"t{next(_ctr)}")
            nc.sync.dma_start(ha_sb, d_ha[l])

            if variant in ("dmabig2", "dmabig4"):
                # split the big transfer across multiple DGE queues
                if li == 0:
                    sink = singles.tile([128, 1], dt.float32, name="sink")
                    WALL = 9556
                    d_wall = nc.dram_tensor("wall", [L, 128, WALL], dt.float16,
                                            kind="ExternalInput")
                w = wp16.tile([128, 9556], dt.float16, tag="wall",
                              name=f"t{next(_ctr)}")
                if variant == "dmabig2":
                    nc.sync.dma_start(w[:, :4778], d_wall[l][:, :4778])
                    nc.gpsimd.dma_start(w[:, 4778:], d_wall[l][:, 4778:])
                else:
                    nc.sync.dma_start(w[:, :3185], d_wall[l][:, :3185])
                    nc.scalar.dma_start(w[:, 3185:6370], d_wall[l][:, 3185:6370])
                    nc.gpsimd.dma_start(w[:, 6370:], d_wall[l][:, 6370:])
                nc.vector.tensor_copy(out=sink, in_=w[:, 0:1])
                continue

            if variant in ("dmabig", "dmabig_sw"):
                # single huge DMA per layer, same total bytes as all weights
                if li == 0:
                    sink = singles.tile([128, 1], dt.float32, name="sink")
                    WALL = 9556  # fp16 elems/partition ~= 19.1MB total
                    d_wall = nc.dram_tensor("wall", [L, 128, WALL], dt.float16,
                                            kind="ExternalInput")
                w = wp16.tile([128, 9556], dt.float16, tag="wall",
                              name=f"t{next(_ctr)}")
                if variant == "dmabig_sw":
                    nc.gpsimd.dma_start(w, d_wall[l])
                else:
                    nc.sync.dma_start(w, d_wall[l])
                nc.vector.tensor_copy(out=sink, in_=w[:, 0:1])
                continue

            if dma_only:
                if li == 0:
                    sink = singles.tile([128, 1], dt.float32, name="sink")
                for dram, kt_, m in ((d_wkt, KT, MQ), (d_wvt, KT, D),
                                     (d_wka, KT, MQ), (d_wva, KT, D),
                                     (d_wq, KT, MQ), (d_wks, KT, MQ),
                                     (d_wvs, KT, D), (d_wo, KTO, D)):
                    w = load_w(dram, l, kt_, m)
                    nc.vector.tensor_copy(out=sink, in_=w[:, 0, 0:1])
                for half in (0, 1):
                    w = load_w32(d_wf, l, KT, half)
                    nc.vector.tensor_copy(out=sink, in_=w[:, 0, 0:1])
                nc.vector.tensor_copy(out=sink, in_=ht_sb[:, 0, 0:1])
                nc.vector.tensor_copy(out=sink, in_=ha_sb[:, 0, 0:1])
                continue

            # fp16 shadow of the fp32 residual stream (on ACT, DVE is busier)
            x16 = xp.tile([128, KT, T], dt.float16, tag="x16",
                          name=f"t{next(_ctr)}")
            nc.scalar.activation(x16, x_sb, AF.Copy)

            # ---- kv path ----
            w = load_w(d_wkt, l, KT, MQ)
            kt_ro = proj_qk(w, ht_sb, NVIS, B_KT, ct, stg, bsb)
            tap("kt_ro", kt_ro, l)
            w = load_w(d_wvt, l, KT, D)
            vt_sb = proj_v(w, ht_sb, 4, 128, "t", 2, bvsb)
            tap("vt", vt_sb, l)
            w = load_w(d_wka, l, KT, MQ)
            ka_ro = proj_qk(w, ha_sb, NA, B_KA, ca, sa, bsb)
            w = load_w(d_wva, l, KT, D)
            va_sb = proj_v(w, ha_sb, 1, NA, "a", 1, bvsb)

            # ---- x projections ----
            w = load_w(d_wq, l, KT, MQ)
            q_ro = proj_qk(w, x16, T, B_Q, cq, sq, bsb)
            tap("q_ro", q_ro, l)
            w = load_w(d_wks, l, KT, MQ)
            ks_ro = proj_qk(w, x16, T, B_KS, cq, sq, bsb)
            w = load_w(d_wvs, l, KT, D)
            vs_sb = proj_v(w, x16, 1, T, "s", 0, bvsb)

            # ---- attention (scores transposed: [keys, queries]) ----
            NT = NH * T  # 448
            ex_s = att.tile([T, NT], dt.float16, tag="exs",
                            name=f"t{next(_ctr)}")
            ex_a = att.tile([NA, NT], dt.float16, tag="exa",
                            name=f"t{next(_ctr)}")
            ex_t = att.tile([128, 4, NT], dt.float16, tag="ext",
                            name=f"t{next(_ctr)}")

            ps = ps1.tile([T, 512], dt.float32, tag="ps1",
                          name=f"t{next(_ctr)}")[:, :NT]
            for h in range(NH):
                nc.tensor.matmul(ps[:, T * h:T * h + T], ks_ro[:, h, :],
                                 q_ro[:, h, :], start=True, stop=True)
            nc.scalar.activation(ex_s, ps, AF.Exp)
            tap("ex_s", ex_s, l)
            ps = ps1.tile([NA, 512], dt.float32, tag="ps1",
                          name=f"t{next(_ctr)}")[:, :NT]
            for h in range(NH):
                nc.tensor.matmul(ps[:, T * h:T * h + T], ka_ro[:, h, :],
                                 q_ro[:, h, :], start=True, stop=True)
            nc.scalar.activation(ex_a, ps, AF.Exp)
            for m in range(4):
                ps = ps1.tile([128, 512], dt.float32, tag="ps1",
                              name=f"t{next(_ctr)}")[:, :NT]
                for h in range(NH):
                    nc.tensor.matmul(ps[:, T * h:T * h + T],
                                     kt_ro[:, h, 128 * m:128 * m + 128],
                                     q_ro[:, h, :], start=True, stop=True)
                nc.scalar.activation(ex_t[:, m, :], ps, AF.Exp)

            lps = ps1.tile([1, 512], dt.float32, tag="ps1",
                           name=f"t{next(_ctr)}")[:, :NT]
            nc.tensor.matmul(lps, ones_h[:T], ex_s, start=True, stop=False)
            nc.tensor.matmul(lps, ones_h[:NA], ex_a, start=False, stop=False)
            for m in range(4):
                nc.tensor.matmul(lps, ones_h, ex_t[:, m, :],
                                 start=False, stop=(m == 3))
            linv = st.tile([1, NT], dt.float32, tag="linv",
                           name=f"t{next(_ctr)}")
            nc.vector.reciprocal(linv, lps)
            tap("linv", linv, l)
            # linv_b carries DS so that o16 = o * DS; wo carries WSCALE.
            lrep_ps = ps1.tile([128, 512], dt.float32, tag="ps1",
                               name=f"t{next(_ctr)}")[:, :NT]
            nc.tensor.matmul(lrep_ps, ds_row, linv, start=True, stop=True)
            linv_b = att.tile([128, NT], dt.float32, tag="linvb",
                              name=f"t{next(_ctr)}")
            nc.vector.tensor_copy(out=linv_b, in_=lrep_ps)

            o16 = att.tile([128, NH, T], dt.float16, tag="opad",
                           name=f"t{next(_ctr)}")
            nc.vector.memset(o16[96:128], 0.0)
            for h in range(NH):
                hs = slice(HD * h, HD * h + HD)
                ops = ps1.tile([HD, 512], dt.float32, tag="ps1",
                               name=f"t{next(_ctr)}")[:, :T]
                nc.tensor.matmul(ops, vs_sb[:T, 0, hs],
                                 ex_s[:, T * h:T * h + T],
                                 start=True, stop=False)
                nc.tensor.matmul(ops, va_sb[:NA, 0, hs],
                                 ex_a[:, T * h:T * h + T],
                                 start=False, stop=False)
                for m in range(4):
                    nc.tensor.matmul(ops, vt_sb[:, m, hs],
                                     ex_t[:, m, T * h:T * h + T],
                                     start=False, stop=(m == 3))
                nc.vector.tensor_tensor(
                    o16[:HD, h, :], ops, linv_b[:HD, T * h:T * h + T], OP.mult)
            tap("o_pad", o16, l)

            # ---- Wo (fp16, scale pre-folded) + residual ----
            y_sb = yp.tile([128, KT, T], dt.float32, tag="y",
                           name=f"t{next(_ctr)}")
            w = load_w(d_wo, l, KTO, D)
            for mo in range(KT):
                ps = ps1.tile([128, 512], dt.float32, tag="ps1",
                              name=f"t{next(_ctr)}")[:, :T]
                for k in range(KTO):
                    nc.tensor.matmul(ps, w[:, k, 128 * mo:128 * mo + 128],
                                     o16[:, k, :],
                                     start=(k == 0), stop=(k == KTO - 1))
                nc.vector.scalar_tensor_tensor(
                    y_sb[:, mo, :], ps, bsb[:, B_O + mo:B_O + mo + 1],
                    x_sb[:, mo, :], OP.add, OP.add)
            tap("y", y_sb, l)

            # ---- layernorm (fp32) ----
            mps = ps1.tile([1, 512], dt.float32, tag="ps1",
                           name=f"t{next(_ctr)}")[:, :T]
            for k in range(KT):
                nc.tensor.matmul(mps, ones_f, y_sb[:, k, :],
                                 start=(k == 0), stop=(k == KT - 1))
            ysq = yp.tile([128, KT, T], dt.float32, tag="ysq",
                          name=f"t{next(_ctr)}")
            nc.scalar.activation(ysq, y_sb, AF.Square)
            sps = ps1.tile([1, 512], dt.float32, tag="ps1",
                           name=f"t{next(_ctr)}")[:, :T]
            for k in range(KT):
                nc.tensor.matmul(sps, ones_f, ysq[:, k, :],
                                 start=(k == 0), stop=(k == KT - 1))
            mean = st.tile([1, T], dt.float32, tag="mean",
                           name=f"t{next(_ctr)}")
            nc.vector.tensor_scalar_mul(mean, mps, 1.0 / D)
            msq = st.tile([1, T], dt.float32, tag="msq", name=f"t{next(_ctr)}")
            nc.vector.tensor_tensor(msq, mean, mean, OP.mult)
            var = st.tile([1, T], dt.float32, tag="var", name=f"t{next(_ctr)}")
            nc.vector.scalar_tensor_tensor(var, sps, 1.0 / D, msq,
                                           OP.mult, OP.subtract)
            rc = st.tile([1, 2 * T], dt.float32, tag="rc", name=f"t{next(_ctr)}")
            nc.scalar.activation(rc[:, :T], var, AF.Sqrt, bias=eps_t)
            nc.vector.reciprocal(rc[:, :T], rc[:, :T])
            nc.vector.tensor_tensor(rc[:, T:], mean, rc[:, :T], OP.mult)
            rrep_ps = ps1.tile([128, 512], dt.float32, tag="ps1",
                               name=f"t{next(_ctr)}")[:, :2 * T]
            nc.tensor.matmul(rrep_ps, ones_row, rc, start=True, stop=True)
            rc_b = st.tile([128, 2 * T], dt.float32, tag="rcb",
                           name=f"t{next(_ctr)}")
            nc.vector.tensor_copy(out=rc_b, in_=rrep_ps)
            lnf = yp.tile([128, KT, T], dt.float32, tag="lnf",
                          name=f"t{next(_ctr)}")
            for k in range(KT):
                t1 = tmp.tile([128, T], dt.float32, tag="lt1",
                              name=f"t{next(_ctr)}")
                nc.vector.tensor_tensor(t1, y_sb[:, k, :], rc_b[:, :T], OP.mult)
                t2 = tmp.tile([128, T], dt.float32, tag="lt2",
                              name=f"t{next(_ctr)}")
                nc.vector.tensor_tensor(t2, t1, rc_b[:, T:], OP.subtract)
                nc.vector.tensor_scalar(lnf[:, k, :], t2,
                                        bsb[:, B_G + k:B_G + k + 1],
                                        bsb[:, B_B + k:B_B + k + 1],
                                        OP.mult, OP.add)
            tap("ln", lnf, l)

            # ---- Wf (fp32) + relu -> new x ----
            x_new = xp.tile([128, KT, T], dt.float32, tag="x",
                            name=f"t{next(_ctr)}")
            for half, mos in ((0, range(4)), (1, range(4, KT))):
                w32 = load_w32(d_wf, l, KT, half)
                for mo in mos:
                    mc = mo * 128 - half * 512
                    ps = ps1.tile([128, 512], dt.float32, tag="ps1",
                                  name=f"t{next(_ctr)}")[:, :T]
                    for k in range(KT):
                        nc.tensor.matmul(ps, w32[:, k, mc:mc + 128],
                                         lnf[:, k, :],
                                         start=(k == 0), stop=(k == KT - 1))
                    nc.scalar.activation(x_new[:, mo, :], ps, AF.Relu,
                                         bias=bsb[:, B_F + mo:B_F + mo + 1])
            x_sb = x_new
            if d_xdbg is not None and li == l:
                nc.sync.dma_start(d_xdbg[l], x_sb)

        # ---- final layernorm + Wout (fp32) ----
        mps = ps1.tile([1, 512], dt.float32, tag="ps1",
                       name=f"t{next(_ctr)}")[:, :T]
        for k in range(KT):
            nc.tensor.matmul(mps, ones_f, x_sb[:, k, :],
                             start=(k == 0), stop=(k == KT - 1))
        ysq = yp.tile([128, KT, T], dt.float32, tag="ysq", name=f"t{next(_ctr)}")
        nc.scalar.activation(ysq, x_sb, AF.Square)
        sps = ps1.tile([1, 512], dt.float32, tag="ps1",
                       name=f"t{next(_ctr)}")[:, :T]
        for k in range(KT):
            nc.tensor.matmul(sps, ones_f, ysq[:, k, :],
                             start=(k == 0), stop=(k == KT - 1))
        mean = st.tile([1, T], dt.float32, tag="mean", name=f"t{next(_ctr)}")
        nc.vector.tensor_scalar_mul(mean, mps, 1.0 / D)
        msq = st.tile([1, T], dt.float32, tag="msq", name=f"t{next(_ctr)}")
        nc.vector.tensor_tensor(msq, mean, mean, OP.mult)
        var = st.tile([1, T], dt.float32, tag="var", name=f"t{next(_ctr)}")
        nc.vector.scalar_tensor_tensor(var, sps, 1.0 / D, msq,
                                       OP.mult, OP.subtract)
        rc = st.tile([1, 2 * T], dt.float32, tag="rc", name=f"t{next(_ctr)}")
        nc.scalar.activation(rc[:, :T], var, AF.Sqrt, bias=eps_t)
        nc.vector.reciprocal(rc[:, :T], rc[:, :T])
        nc.vector.tensor_tensor(rc[:, T:], mean, rc[:, :T], OP.mult)
        rrep_ps = ps1.tile([128, 512], dt.float32, tag="ps1",
                           name=f"t{next(_ctr)}")[:, :2 * T]
        nc.tensor.matmul(rrep_ps, ones_row, rc, start=True, stop=True)
        rc_b = st.tile([128, 2 * T], dt.float32, tag="rcb", name=f"t{next(_ctr)}")
        nc.vector.tensor_copy(out=rc_b, in_=rrep_ps)
        lnf = yp.tile([128, KT, T], dt.float32, tag="lnf", name=f"t{next(_ctr)}")
        for k in range(KT):
            t1 = tmp.tile([128, T], dt.float32, tag="lt1", name=f"t{next(_ctr)}")
            nc.vector.tensor_tensor(t1, x_sb[:, k, :], rc_b[:, :T], OP.mult)
            t2 = tmp.tile([128, T], dt.float32, tag="lt2", name=f"t{next(_ctr)}")
            nc.vector.tensor_tensor(t2, t1, rc_b[:, T:], OP.subtract)
            nc.vector.tensor_scalar(lnf[:, k, :], t2,
                                    fin[:, k:k + 1], fin[:, 7 + k:8 + k],
                                    OP.mult, OP.add)
        woutsb = wp32.tile([128, KT, VOCAB], dt.float32, tag="w32",
                           name="woutsb")
        nc.sync.dma_start(woutsb, d_wout[:])
        out_sb = yp.tile([128, 2, T], dt.float32, tag="outsb",
                         name=f"t{next(_ctr)}")
        for mo in range(2):
            ps = ps1.tile([128, 512], dt.float32, tag="ps1",
                          name=f"t{next(_ctr)}")[:, :T]
            for k in range(KT):
                nc.tensor.matmul(ps, woutsb[:, k, 128 * mo:128 * mo + 128],
                                 lnf[:, k, :],
                                 start=(k == 0), stop=(k == KT - 1))
            nc.vector.tensor_scalar_add(out_sb[:, mo, :], ps,
                                        fin[:, 14 + mo:15 + mo])
        nc.sync.dma_start(d_out[:], out_sb)

    nc.compile()
    return nc


_PROG_CACHE = {}


def _get_program(L, xdbg=False, variant=None):
    key = (L, xdbg, variant)
    if key not in _PROG_CACHE:
        _PROG_CACHE[key] = build_program(L, xdbg, variant)
    return _PROG_CACHE[key]


def run(inputs, L=L_FULL, xdbg=False):
    from concourse.bass_utils import run_bass_kernel_spmd
    nc = _get_program(L, xdbg)
    shared, g = prep_shared(inputs, L)
    in_maps = []
    for b in range(NCORES):
        m = dict(shared)
        m.update(prep_core(g, b, L))
        in_maps.append(m)
    res = run_bass_kernel_spmd(nc, in_maps, core_ids=list(range(NCORES)))
    outs = []
    for r in res.results:
        o = r["out"]                                    # [128, 2, T]
        outs.append(np.ascontiguousarray(o.transpose(2, 1, 0)).reshape(T, VOCAB))
    full = np.stack(outs).astype(F32)                   # [B, T, VOCAB]
    if xdbg:
        return full, res.results
    return full


def kernel(**inputs) -> np.ndarray:
    return run(inputs, L=L_FULL)

